# revision 1
# baseline (speedup 1.0000x reference)
"""Trainium2 Bass kernel for AttentionProlongationGNN.

Contract: kernel(**inputs) takes FULL unsharded numpy inputs (keys as in
setup_inputs) and returns the FULL (N, 1) float32 output.

Strategy (8 NeuronCores, SPMD single program):
- Nodes sharded 6250/core (padded to 6272 = 49*128 rows).  Each core keeps its
  h shard and computes Q/K/V shards densely on TensorE; K,V are AllGathered
  (bf16) each layer so every core holds full gather tables in HBM.
- Edges sharded by dst: each core owns edges whose dst lies in its node shard,
  sorted by dst and grouped into 49 dst-blocks of 128 nodes; each block's edge
  list is padded to whole 128-edge tiles (schedule uniform across cores).
- Per tile: batched indirect-DMA row gathers of K[src],V[src] (bf16) and
  Q[dst] (fp32), per-head dot on DVE, edge bias via a K=3 matmul, block-batched
  leaky-relu+exp, then segment-sum via a PE matmul with an on-chip is_equal
  selection matrix, accumulating over the block in PSUM.
- Softmax stabilization max cancels mathematically: aggregate unnormalized
  exp-weighted V plus the exp sums, divide at block drain.
- Dense Wo/Wm/LayerNorm and the output head run per 128-row tile on PE/DVE/ACT.
"""
import sys

if "/opt/trn_rl_repo" not in sys.path:
    sys.path.insert(0, "/opt/trn_rl_repo")

import numpy as np

from concourse import bass, mybir, bacc, tile
from concourse.masks import make_identity
from concourse.bass_utils import run_bass_kernel_spmd

FP = mybir.dt.float32
BF = mybir.dt.bfloat16
I32 = mybir.dt.int32
AF = mybir.ActivationFunctionType
OP = mybir.AluOpType

P = 128
NCORES = 8
H = 256
NH = 8
HD = H // NH
ED = 3
F_IN = 10
L = 3
EPS_LN = 1e-5
HC = H // P            # feature chunks (2)


# ---------------------------------------------------------------- host prep

def prep_edges(edge_index, N):
    """Per-core edge schedule, uniform across cores.

    Returns (tiles_per_block, block_tile_off, T_tot, cores) where each core
    dict has src_rows/qdst_rows int32 [P, T_tot], dstloc fp32 [P, T_tot],
    eattr_sel int64 [T_tot*P] (original edge id or -1).
    """
    nsh = N // NCORES
    blk = (nsh + P - 1) // P
    nsh_pad = blk * P
    src = edge_index[0].astype(np.int64)
    dst = edge_index[1].astype(np.int64)
    core_of = dst // nsh
    counts = np.zeros((NCORES, blk), np.int64)
    per_core = []
    for c in range(NCORES):
        eids = np.where(core_of == c)[0]
        ld = dst[eids] - c * nsh
        b = ld // P
        order = np.argsort(b, kind="stable")
        eids = eids[order]
        b = b[order]
        per_core.append((eids, b))
        counts[c] = np.bincount(b, minlength=blk)
    tiles_per_block = np.maximum(1, -(-counts.max(axis=0) // P)).astype(np.int64)
    T_tot = int(tiles_per_block.sum())
    block_tile_off = np.concatenate([[0], np.cumsum(tiles_per_block)])[:-1]

    cores = []
    for c in range(NCORES):
        eids, b = per_core[c]
        src_rows = np.zeros((P, T_tot), np.int32)
        qdst_rows = np.zeros((P, T_tot), np.int32)
        dstloc = np.full((P, T_tot), -1.0, np.float32)
        esel = np.full(T_tot * P, -1, np.int64)
        for blki in range(blk):
            be = eids[b == blki]
            t0 = block_tile_off[blki]
            n = len(be)
            tt = t0 + np.arange(n) // P
            pp = np.arange(n) % P
            s = src[be]
            src_rows[pp, tt] = (s // nsh) * nsh_pad + (s % nsh)
            ldl = dst[be] - c * nsh - blki * P
            qdst_rows[pp, tt] = blki * P + ldl
            dstloc[pp, tt] = ldl.astype(np.float32)
            esel[tt * P + pp] = be
        cores.append(dict(src_rows=src_rows, qdst_rows=qdst_rows,
                          dstloc=dstloc, esel=esel))
    return tiles_per_block, block_tile_off, T_tot, cores


# ------------------------------------------------------------- device build

def build_program(N, T_tot, tiles_per_block, block_tile_off):
    nsh = N // NCORES
    blk = (nsh + P - 1) // P
    nsh_pad = blk * P
    rg = [list(range(NCORES))]

    nc = bacc.Bacc("TRN2", target_bir_lowering=False, debug=False,
                   num_devices=NCORES)

    # ---- I/O
    xT = nc.dram_tensor("xT", [F_IN, nsh_pad], FP, kind="ExternalInput")
    srcrows = nc.dram_tensor("srcrows", [P, T_tot], I32, kind="ExternalInput")
    qdstrows = nc.dram_tensor("qdstrows", [P, T_tot], I32, kind="ExternalInput")
    dstloc = nc.dram_tensor("dstloc", [P, T_tot], FP, kind="ExternalInput")
    eattrT = nc.dram_tensor("eattrT", [ED, T_tot * P], FP, kind="ExternalInput")
    iota_in = nc.dram_tensor("iota_in", [P, P], FP, kind="ExternalInput")
    w_in = nc.dram_tensor("w_in", [F_IN, H], FP, kind="ExternalInput")
    b_in = nc.dram_tensor("b_in", [1, H], FP, kind="ExternalInput")
    wqs = nc.dram_tensor("wqs", [L, H, H], FP, kind="ExternalInput")
    wk = nc.dram_tensor("wk", [L, H, H], FP, kind="ExternalInput")
    wv = nc.dram_tensor("wv", [L, H, H], FP, kind="ExternalInput")
    we = nc.dram_tensor("we", [L, ED, NH], FP, kind="ExternalInput")
    wo = nc.dram_tensor("wo", [L, H, H], FP, kind="ExternalInput")
    bo = nc.dram_tensor("bo", [L, H], FP, kind="ExternalInput")
    wm = nc.dram_tensor("wm", [L, 2 * H, H], FP, kind="ExternalInput")
    bm = nc.dram_tensor("bm", [L, H], FP, kind="ExternalInput")
    gam = nc.dram_tensor("gam", [L, H], FP, kind="ExternalInput")
    bet = nc.dram_tensor("bet", [L, H], FP, kind="ExternalInput")
    wh1 = nc.dram_tensor("wh1", [H, P], FP, kind="ExternalInput")
    bh1 = nc.dram_tensor("bh1", [1, P], FP, kind="ExternalInput")
    wh2 = nc.dram_tensor("wh2", [P, 1], FP, kind="ExternalInput")
    bh2 = nc.dram_tensor("bh2", [1, 1], FP, kind="ExternalInput")
    y = nc.dram_tensor("y", [nsh_pad, 1], FP, kind="ExternalOutput")

    with tile.TileContext(nc) as tc:
        with (
            tc.tile_pool(name="sbw", bufs=1) as sbw,       # persistent weights
            tc.tile_pool(name="sbd", bufs=2) as sbd,       # dense working tiles
            tc.tile_pool(name="sbg", bufs=2) as sbg,       # per-block gather tiles
            tc.tile_pool(name="sbe", bufs=3) as sbe,       # per-tile edge working
            tc.tile_pool(name="dram", bufs=1, space="DRAM") as dram,
            tc.tile_pool(name="p_big", bufs=2, space="PSUM") as p_big,
            tc.tile_pool(name="p_tr", bufs=2, space="PSUM") as p_tr,
            tc.tile_pool(name="p_acc", bufs=2, space="PSUM") as p_acc,
            tc.tile_pool(name="p_sm", bufs=2, space="PSUM") as p_sm,
        ):
            # ---- persistent SBUF constants
            ident = sbw.tile([P, P], FP)
            make_identity(nc, ident[:])
            iota_sb = sbw.tile([P, P], FP)
            nc.sync.dma_start(iota_sb[:], iota_in[:])
            ones1 = sbw.tile([1, P], FP)
            nc.vector.memset(ones1[:], 1.0)
            eps_col = sbw.tile([P, 1], FP)
            nc.vector.memset(eps_col[:], EPS_LN)

            w_in_sb = sbw.tile([F_IN, H], FP)
            nc.sync.dma_start(w_in_sb[:], w_in[:])
            b_in_sb = sbw.tile([1, H], FP)
            nc.sync.dma_start(b_in_sb[:], b_in[:])
            wh1_sb = [sbw.tile([P, P], FP, name=f"wh1_{kc}", tag=f"wh1_{kc}")
                      for kc in range(HC)]
            for kc in range(HC):
                nc.sync.dma_start(wh1_sb[kc][:], wh1[kc * P:(kc + 1) * P, :])
            bh1_sb = sbw.tile([1, P], FP)
            nc.sync.dma_start(bh1_sb[:], bh1[:])
            wh2_sb = sbw.tile([P, 1], FP)
            nc.sync.dma_start(wh2_sb[:], wh2[:])
            bh2_sb = sbw.tile([1, 1], FP)
            nc.sync.dma_start(bh2_sb[:], bh2[:])

            def load_w_chunks(t, l):  # [L, H, H] -> list of [P, H] chunk tiles
                out = []
                for kc in range(HC):
                    s = sbw.tile([P, H], FP, name=f"w_{t.name}_{l}_{kc}", tag=f"w_{t.name}_{l}_{kc}")
                    nc.sync.dma_start(s[:], t[l, kc * P:(kc + 1) * P, :])
                    out.append(s)
                return out

            wqs_sb = [load_w_chunks(wqs, l) for l in range(L)]
            wk_sb = [load_w_chunks(wk, l) for l in range(L)]
            wv_sb = [load_w_chunks(wv, l) for l in range(L)]
            wo_sb = [load_w_chunks(wo, l) for l in range(L)]
            wm_sb = []
            for l in range(L):
                chunks = []
                for kc in range(2 * HC):
                    s = sbw.tile([P, H], FP, name=f"w_wm_{l}_{kc}", tag=f"w_wm_{l}_{kc}")
                    nc.sync.dma_start(s[:], wm[l, kc * P:(kc + 1) * P, :])
                    chunks.append(s)
                wm_sb.append(chunks)
            we_sb = []
            row_l = {}
            for nm, t in [("bo", bo), ("bm", bm), ("gam", gam), ("bet", bet)]:
                row_l[nm] = []
                for l in range(L):
                    s = sbw.tile([1, H], FP, name=f"row_{nm}_{l}", tag=f"row_{nm}_{l}")
                    nc.sync.dma_start(s[:], t[l:l + 1, :])
                    row_l[nm].append(s)
            for l in range(L):
                s = sbw.tile([ED, NH], FP, name=f"we_{l}", tag=f"we_{l}")
                nc.sync.dma_start(s[:], we[l])
                we_sb.append(s)

            # replicated gamma/beta [P, H] per layer (built on first use)
            gb_rep = {}

            # ---- internal DRAM
            hdr = dram.tile([nsh_pad, H], FP)
            htr = dram.tile([HC * P, nsh_pad], FP)
            qtab = dram.tile([nsh_pad, H], BF)
            kcb = dram.tile([nsh_pad, H], BF)
            vcb = dram.tile([nsh_pad, H], BF)
            kfull_l = [dram.tile([NCORES * nsh_pad, H], BF, addr_space="Shared",
                                 name=f"kfull_{l}", tag=f"kfull_{l}") for l in range(L)]
            vfull_l = [dram.tile([NCORES * nsh_pad, H], BF, addr_space="Shared",
                                 name=f"vfull_{l}", tag=f"vfull_{l}") for l in range(L)]
            aggdr = dram.tile([nsh_pad, H], FP)

            # ---------------- helpers
            def bias_outer(ps, row_sb, ncols, stop):
                nc.tensor.matmul(ps[:, 0:ncols], lhsT=ones1[:, 0:ps.shape[0]],
                                 rhs=row_sb[:, 0:ncols], start=False, stop=stop)

            def transpose_to_sb(dst_sb, src_sb_ap):
                pt = p_tr.tile([P, P], FP, name="ptr", tag="ptr")
                nc.tensor.transpose(out=pt[:], in_=src_sb_ap, identity=ident[:])
                nc.any.tensor_copy(dst_sb, pt[:])

            def store_hT(h_sb, r):
                # write h rows tile to htr (feature-major) via 2 transposes
                for kc in range(HC):
                    tt = sbd.tile([P, P], FP, name="hT_t", tag="hT_t")
                    transpose_to_sb(tt[:], h_sb[:, kc * P:(kc + 1) * P])
                    nc.sync.dma_start(htr[kc * P:(kc + 1) * P, r * P:(r + 1) * P], tt[:])

            # ---------------- input projection: h0 = relu(x @ W_in + b_in)
            for r in range(blk):
                xt = sbd.tile([F_IN, P], FP, name="xt", tag="xt")
                nc.sync.dma_start(xt[:], xT[:, r * P:(r + 1) * P])
                ps = p_big.tile([P, H], FP, name="pbig", tag="pbig")
                nc.tensor.matmul(ps[:], lhsT=xt[:], rhs=w_in_sb[:], start=True, stop=False)
                bias_outer(ps, b_in_sb, H, stop=True)
                h_sb = sbd.tile([P, H], FP, name="h_new", tag="h_new")
                nc.scalar.activation(h_sb[:], ps[:], AF.Relu)
                nc.sync.dma_start(hdr[r * P:(r + 1) * P, :], h_sb[:])
                store_hT(h_sb, r)

            # ---------------- layers
            for l in range(L):
                # gamma/beta replicated
                g_rep = sbw.tile([P, H], FP, name=f"grep_{l}", tag=f"grep_{l}")
                b_rep = sbw.tile([P, H], FP, name=f"brep_{l}", tag=f"brep_{l}")
                for dst_rep, row in [(g_rep, row_l["gam"][l]), (b_rep, row_l["bet"][l])]:
                    pr = p_big.tile([P, H], FP, name="pbig", tag="pbig")
                    nc.tensor.matmul(pr[:], lhsT=ones1[:], rhs=row[:], start=True, stop=True)
                    nc.any.tensor_copy(dst_rep[:], pr[:])

                # ---- dense QKV per row tile
                for r in range(blk):
                    ht = []
                    for kc in range(HC):
                        t = sbd.tile([P, P], FP, name="ht_in", tag="ht_in")
                        nc.sync.dma_start(t[:], htr[kc * P:(kc + 1) * P, r * P:(r + 1) * P])
                        ht.append(t)
                    for w_chunks, dst_dram, dt in (
                        (wqs_sb[l], qtab, BF),
                        (wk_sb[l], kcb, BF),
                        (wv_sb[l], vcb, BF),
                    ):
                        ps = p_big.tile([P, H], FP, name="pbig", tag="pbig")
                        for kc in range(HC):
                            nc.tensor.matmul(ps[:], lhsT=ht[kc][:], rhs=w_chunks[kc][:],
                                             start=(kc == 0), stop=(kc == HC - 1))
                        o = sbd.tile([P, H], dt, name=f"qkv_out_{dt}", tag=f"qkv_out_{dt}")
                        nc.any.tensor_copy(o[:], ps[:])
                        nc.sync.dma_start(dst_dram[r * P:(r + 1) * P, :], o[:])

                # ---- allgather K, V
                kfull, vfull = kfull_l[l], vfull_l[l]
                nc.gpsimd.collective_compute("AllGather", OP.bypass,
                                             ins=[kcb[:].opt()], outs=[kfull[:].opt()],
                                             replica_groups=rg)
                nc.gpsimd.collective_compute("AllGather", OP.bypass,
                                             ins=[vcb[:].opt()], outs=[vfull[:].opt()],
                                             replica_groups=rg)

                # ---- edge phase
                for b in range(blk):
                    T_b = int(tiles_per_block[b])
                    off = int(block_tile_off[b])
                    idxk = sbg.tile([P, T_b], I32, name="idxk", tag="idxk")
                    nc.sync.dma_start(idxk[:], srcrows[:, off:off + T_b])
                    idxq = sbg.tile([P, T_b], I32, name="idxq", tag="idxq")
                    nc.sync.dma_start(idxq[:], qdstrows[:, off:off + T_b])
                    dl = sbg.tile([P, T_b], FP, name="dl", tag="dl")
                    nc.sync.dma_start(dl[:], dstloc[:, off:off + T_b])
                    ea = sbg.tile([ED, T_b * P], FP, name="ea", tag="ea")
                    nc.sync.dma_start(ea[:], eattrT[:, off * P:(off + T_b) * P])

                    # HW honours one dynamic offset per partition, so gather
                    # one 128-row tile per indirect DMA ([128,1] offsets).
                    kg = sbg.tile([P, T_b, H], BF, name="kg", tag="kg")
                    vg = sbg.tile([P, T_b, H], BF, name="vg", tag="vg")
                    qg = sbg.tile([P, T_b, H], BF, name="qg", tag="qg")
                    for t in range(T_b):
                        nc.gpsimd.indirect_dma_start(
                            out=kg[:, t, :], out_offset=None, in_=kfull[:],
                            in_offset=bass.IndirectOffsetOnAxis(ap=idxk[:, t:t + 1], axis=0))
                        nc.gpsimd.indirect_dma_start(
                            out=vg[:, t, :], out_offset=None, in_=vfull[:],
                            in_offset=bass.IndirectOffsetOnAxis(ap=idxk[:, t:t + 1], axis=0))
                        nc.gpsimd.indirect_dma_start(
                            out=qg[:, t, :], out_offset=None, in_=qtab[:],
                            in_offset=bass.IndirectOffsetOnAxis(ap=idxq[:, t:t + 1], axis=0))

                    dots = sbg.tile([P, T_b * NH], FP, name="dots", tag="dots")
                    biasp = p_sm.tile([P, T_b * NH], FP, name="psm", tag="psm")
                    for t in range(T_b):
                        nc.tensor.matmul(biasp[:, t * NH:(t + 1) * NH],
                                         lhsT=ea[:, t * P:(t + 1) * P],
                                         rhs=we_sb[l][:], start=True, stop=True)
                        qk = sbe.tile([P, H], FP, name="qk", tag="qk")
                        nc.vector.tensor_tensor(qk[:], qg[:, t, :], kg[:, t, :], op=OP.mult)
                        nc.vector.reduce_sum(
                            dots[:, t * NH:(t + 1) * NH].rearrange("p (h o) -> p h o", o=1),
                            qk[:].rearrange("p (h d) -> p h d", d=HD),
                            axis=mybir.AxisListType.X)
                    lg = sbg.tile([P, T_b * NH], FP, name="lg", tag="lg")
                    nc.vector.scalar_tensor_tensor(lg[:], in0=dots[:], scalar=1.0,
                                                   in1=biasp[:], op0=OP.mult, op1=OP.add)
                    lg2 = sbg.tile([P, T_b * NH], FP, name="lg2", tag="lg2")
                    nc.vector.scalar_tensor_tensor(lg2[:], in0=lg[:], scalar=0.2,
                                                   in1=lg[:], op0=OP.mult, op1=OP.max)
                    aexp = sbg.tile([P, T_b * NH], FP, name="aexp", tag="aexp")
                    nc.scalar.activation(aexp[:], lg2[:], AF.Exp)

                    acc = p_acc.tile([P, H + NH], FP, name="pacc", tag="pacc")
                    for t in range(T_b):
                        rhs = sbe.tile([P, H + NH], FP, name="rhs", tag="rhs")
                        nc.vector.tensor_tensor(
                            rhs[:, 0:H].rearrange("p (h d) -> p h d", d=HD),
                            vg[:, t, :].rearrange("p (h d) -> p h d", d=HD),
                            aexp[:, t * NH:(t + 1) * NH]
                                .rearrange("p (h o) -> p h o", o=1)
                                .to_broadcast([P, NH, HD]),
                            op=OP.mult)
                        nc.any.tensor_copy(rhs[:, H:H + NH], aexp[:, t * NH:(t + 1) * NH])
                        m = sbe.tile([P, P], FP, name="m", tag="m")
                        nc.vector.tensor_tensor(
                            m[:], dl[:, t:t + 1].to_broadcast([P, P]), iota_sb[:],
                            op=OP.is_equal)
                        nc.tensor.matmul(acc[:], lhsT=m[:], rhs=rhs[:],
                                         start=(t == 0), stop=(t == T_b - 1))
                    # drain block: normalize and store agg rows
                    ssum = sbe.tile([P, NH], FP, name="ssum", tag="ssum")
                    nc.vector.tensor_scalar_max(ssum[:], acc[:, H:H + NH], 1e-12)
                    rs = sbe.tile([P, NH], FP, name="rs", tag="rs")
                    nc.vector.reciprocal(rs[:], ssum[:])
                    aggn = sbe.tile([P, H], FP, name="aggn", tag="aggn")
                    nc.vector.tensor_tensor(
                        aggn[:].rearrange("p (h d) -> p h d", d=HD),
                        acc[:, 0:H].rearrange("p (h d) -> p h d", d=HD),
                        rs[:].rearrange("p (h o) -> p h o", o=1).to_broadcast([P, NH, HD]),
                        op=OP.mult)
                    nc.sync.dma_start(aggdr[b * P:(b + 1) * P, :], aggn[:])

                # ---- dense post: Wo, Wm, residual + LN
                for r in range(blk):
                    agg_sb = sbd.tile([P, H], FP, name="agg_in", tag="agg_in")
                    nc.sync.dma_start(agg_sb[:], aggdr[r * P:(r + 1) * P, :])
                    aggT = []
                    for kc in range(HC):
                        t = sbd.tile([P, P], FP, name="aggT", tag="aggT")
                        transpose_to_sb(t[:], agg_sb[:, kc * P:(kc + 1) * P])
                        aggT.append(t)
                    ps = p_big.tile([P, H], FP, name="pbig", tag="pbig")
                    for kc in range(HC):
                        nc.tensor.matmul(ps[:], lhsT=aggT[kc][:], rhs=wo_sb[l][kc][:],
                                         start=(kc == 0), stop=False)
                    bias_outer(ps, row_l["bo"][l], H, stop=True)
                    awo = sbd.tile([P, H], FP, name="awo", tag="awo")
                    nc.any.tensor_copy(awo[:], ps[:])
                    awoT = []
                    for kc in range(HC):
                        t = sbd.tile([P, P], FP, name="awoT", tag="awoT")
                        transpose_to_sb(t[:], awo[:, kc * P:(kc + 1) * P])
                        awoT.append(t)
                    ht = []
                    for kc in range(HC):
                        t = sbd.tile([P, P], FP, name="ht_in2", tag="ht_in2")
                        nc.sync.dma_start(t[:], htr[kc * P:(kc + 1) * P, r * P:(r + 1) * P])
                        ht.append(t)
                    ps2 = p_big.tile([P, H], FP, name="pbig", tag="pbig")
                    for kc in range(HC):
                        nc.tensor.matmul(ps2[:], lhsT=ht[kc][:], rhs=wm_sb[l][kc][:],
                                         start=(kc == 0), stop=False)
                    for kc in range(HC):
                        nc.tensor.matmul(ps2[:], lhsT=awoT[kc][:], rhs=wm_sb[l][HC + kc][:],
                                         start=False, stop=False)
                    bias_outer(ps2, row_l["bm"][l], H, stop=True)
                    upd = sbd.tile([P, H], FP, name="upd", tag="upd")
                    nc.scalar.activation(upd[:], ps2[:], AF.Relu)

                    h_sb = sbd.tile([P, H], FP, name="h_in", tag="h_in")
                    nc.sync.dma_start(h_sb[:], hdr[r * P:(r + 1) * P, :])
                    tt = sbd.tile([P, H], FP, name="resid", tag="resid")
                    nc.vector.tensor_tensor(tt[:], h_sb[:], upd[:], op=OP.add)
                    mu_r = sbd.tile([P, 1], FP, name="mu_r", tag="mu_r")
                    nc.vector.reduce_sum(mu_r[:], tt[:], axis=mybir.AxisListType.X)
                    mu = sbd.tile([P, 1], FP, name="mu", tag="mu")
                    nc.vector.tensor_scalar_mul(mu[:], mu_r[:], 1.0 / H)
                    cent = sbd.tile([P, H], FP, name="cent", tag="cent")
                    nc.vector.tensor_scalar_sub(cent[:], tt[:], mu[:])
                    sq = sbd.tile([P, H], FP, name="sq", tag="sq")
                    nc.scalar.activation(sq[:], cent[:], AF.Square)
                    ssq = sbd.tile([P, 1], FP, name="ssq", tag="ssq")
                    nc.vector.reduce_sum(ssq[:], sq[:], axis=mybir.AxisListType.X)
                    var = sbd.tile([P, 1], FP, name="var", tag="var")
                    nc.vector.tensor_scalar_mul(var[:], ssq[:], 1.0 / H)
                    sd = sbd.tile([P, 1], FP, name="sd", tag="sd")
                    nc.scalar.activation(sd[:], var[:], AF.Sqrt, bias=eps_col[:])
                    rstd = sbd.tile([P, 1], FP, name="rstd", tag="rstd")
                    nc.vector.reciprocal(rstd[:], sd[:])
                    normed = sbd.tile([P, H], FP, name="normed", tag="normed")
                    nc.vector.tensor_scalar_mul(normed[:], cent[:], rstd[:])
                    hg = sbd.tile([P, H], FP, name="hg", tag="hg")
                    nc.vector.tensor_tensor(hg[:], normed[:], g_rep[:], op=OP.mult)
                    h_new = sbd.tile([P, H], FP, name="h_new", tag="h_new")
                    nc.vector.tensor_tensor(h_new[:], hg[:], b_rep[:], op=OP.add)
                    nc.sync.dma_start(hdr[r * P:(r + 1) * P, :], h_new[:])
                    store_hT(h_new, r)

            # ---------------- head
            for r in range(blk):
                ht = []
                for kc in range(HC):
                    t = sbd.tile([P, P], FP, name="ht_hd", tag="ht_hd")
                    nc.sync.dma_start(t[:], htr[kc * P:(kc + 1) * P, r * P:(r + 1) * P])
                    ht.append(t)
                ps = p_big.tile([P, P], FP, name="pbig", tag="pbig")
                for kc in range(HC):
                    nc.tensor.matmul(ps[:], lhsT=ht[kc][:], rhs=wh1_sb[kc][:],
                                     start=(kc == 0), stop=False)
                bias_outer(ps, bh1_sb, P, stop=True)
                t1 = sbd.tile([P, P], FP, name="t1", tag="t1")
                nc.scalar.activation(t1[:], ps[:], AF.Relu)
                t1T = sbd.tile([P, P], FP, name="t1T", tag="t1T")
                transpose_to_sb(t1T[:], t1[:])
                ps2 = p_sm.tile([P, 1], FP, name="psm", tag="psm")
                nc.tensor.matmul(ps2[:], lhsT=t1T[:], rhs=wh2_sb[:], start=True, stop=False)
                bias_outer(ps2, bh2_sb, 1, stop=True)
                yt = sbd.tile([P, 1], FP, name="yt", tag="yt")
                nc.any.tensor_copy(yt[:], ps2[:])
                nc.sync.dma_start(y[r * P:(r + 1) * P, :], yt[:])

    nc.compile()
    return nc


# ------------------------------------------------------------------ driver

def make_in_maps(inputs, tiles_per_block, block_tile_off, T_tot, cores, N):
    nsh = N // NCORES
    blk = (nsh + P - 1) // P
    nsh_pad = blk * P
    x = np.asarray(inputs["x"], np.float32)
    edge_attr = np.asarray(inputs["edge_attr"], np.float32)
    scale = HD ** -0.5
    common = {
        "iota_in": np.tile(np.arange(P, dtype=np.float32)[None, :], (P, 1)),
        "w_in": np.asarray(inputs["W_in"], np.float32),
        "b_in": np.asarray(inputs["b_in"], np.float32).reshape(1, H),
        "wqs": np.asarray(inputs["Wq"], np.float32) * scale,
        "wk": np.asarray(inputs["Wk"], np.float32),
        "wv": np.asarray(inputs["Wv"], np.float32),
        "we": np.asarray(inputs["We"], np.float32),
        "wo": np.asarray(inputs["Wo"], np.float32),
        "bo": np.asarray(inputs["bo"], np.float32),
        "wm": np.asarray(inputs["Wm"], np.float32),
        "bm": np.asarray(inputs["bm"], np.float32),
        "gam": np.asarray(inputs["gamma"], np.float32),
        "bet": np.asarray(inputs["beta"], np.float32),
        "wh1": np.asarray(inputs["W_h1"], np.float32),
        "bh1": np.asarray(inputs["b_h1"], np.float32).reshape(1, P),
        "wh2": np.asarray(inputs["W_h2"], np.float32),
        "bh2": np.asarray(inputs["b_h2"], np.float32).reshape(1, 1),
    }
    in_maps = []
    for c in range(NCORES):
        arr = cores[c]
        xT = np.zeros((F_IN, nsh_pad), np.float32)
        xT[:, :nsh] = x[c * nsh:(c + 1) * nsh].T
        esel = arr["esel"]
        ea = np.zeros((T_tot * P, ED), np.float32)
        valid = esel >= 0
        ea[valid] = edge_attr[esel[valid]]
        m = dict(common)
        m.update({
            "xT": xT,
            "srcrows": arr["src_rows"],
            "qdstrows": arr["qdst_rows"],
            "dstloc": arr["dstloc"],
            "eattrT": np.ascontiguousarray(ea.T),
        })
        in_maps.append(m)
    return in_maps


_BUILD_CACHE = {}
LAST_EXEC_NS = None


def kernel(**inputs) -> np.ndarray:
    global LAST_EXEC_NS
    import os
    edge_index = np.asarray(inputs["edge_index"])
    N = inputs["x"].shape[0]
    nsh = N // NCORES
    blk = (nsh + P - 1) // P
    tiles_per_block, block_tile_off, T_tot, cores = prep_edges(edge_index, N)
    key = (N, T_tot, tuple(tiles_per_block.tolist()))
    if key not in _BUILD_CACHE:
        _BUILD_CACHE[key] = build_program(N, T_tot, tiles_per_block, block_tile_off)
    nc = _BUILD_CACHE[key]
    in_maps = make_in_maps(inputs, tiles_per_block, block_tile_off, T_tot, cores, N)
    trace = os.environ.get("KERNEL_TRACE", "0") == "1"
    res = run_bass_kernel_spmd(nc, in_maps, core_ids=list(range(NCORES)),
                               trace=trace)
    if res.exec_time_ns is not None:
        LAST_EXEC_NS = res.exec_time_ns
        tp = res.instructions_and_trace[1] if res.instructions_and_trace else None
        print(f"[kernel] exec_time_ns={res.exec_time_ns} trace={tp}")
    out = np.concatenate([res.results[c]["y"][:nsh] for c in range(NCORES)], 0)
    return out.astype(np.float32)


if __name__ == "__main__":
    # tiny self-check via MultiCoreSim on a small synthetic graph
    import argparse
    parser = argparse.ArgumentParser()
    parser.add_argument("--sim", action="store_true")
    args = parser.parse_args()
    if args.sim:
        from concourse.bass_interp import MultiCoreSim
        rng = np.random.default_rng(0)
        Nl, El = 2048, 8192
        g = lambda *s: (rng.standard_normal(s) * 0.05).astype(np.float32)
        inp = {
            "x": rng.standard_normal((Nl, F_IN)).astype(np.float32),
            "edge_attr": rng.standard_normal((El, ED)).astype(np.float32),
            "W_in": g(F_IN, H), "b_in": np.zeros(H, np.float32),
            "Wq": g(L, H, H), "Wk": g(L, H, H), "Wv": g(L, H, H),
            "We": g(L, ED, NH), "Wo": g(L, H, H),
            "bo": np.zeros((L, H), np.float32),
            "Wm": g(L, 2 * H, H), "bm": np.zeros((L, H), np.float32),
            "gamma": np.ones((L, H), np.float32),
            "beta": np.zeros((L, H), np.float32),
            "W_h1": g(H, H // 2), "b_h1": np.zeros(H // 2, np.float32),
            "W_h2": g(H // 2, 1), "b_h2": np.zeros(1, np.float32),
            "edge_index": rng.integers(0, Nl, size=(2, El)).astype(np.int64),
        }
        tiles_per_block, block_tile_off, T_tot, cores = prep_edges(inp["edge_index"], Nl)
        print(f"sim build: T_tot={T_tot}")
        nc = build_program(Nl, T_tot, tiles_per_block, block_tile_off)
        in_maps = make_in_maps(inp, tiles_per_block, block_tile_off, T_tot, cores, Nl)
        sim = MultiCoreSim(nc, num_cores=NCORES, num_workers=0)
        for c in range(NCORES):
            for k, v in in_maps[c].items():
                sim.cores[c].tensor(k)[:] = v
        sim.simulate(check_with_hw=False)
        nshl = Nl // NCORES
        got = np.concatenate([np.asarray(sim.cores[c].tensor("y"))[:nshl]
                              for c in range(NCORES)], 0)
        # numpy reference
        import prep
        prep.N = Nl; prep.E = El; prep.NSH = nshl
        prep.BLK = (nshl + P - 1) // P; prep.NSH_PAD = prep.BLK * P
        want = prep.np_forward_restructured(inp)
        err = np.abs(got - want)
        print(f"sim maxabs={err.max():.3e} rel={err.max()/np.abs(want).max():.3e}")



# revision 19
# speedup vs baseline: 1.4254x; 1.4254x over previous
"""Trainium2 Bass kernel for AttentionProlongationGNN (optimized).

Contract: kernel(**inputs) takes FULL unsharded numpy inputs (keys as in
setup_inputs) and returns the FULL (N, 1) float32 output.

Strategy (8 NeuronCores, SPMD single program):
- Nodes sharded 6250/core (padded to 6272 = 49*128 rows).  Edges sharded by
  dst core, grouped into 49 dst-blocks of 128 nodes.
- Per layer each core computes K|V for its shard (bf16, one [nsh, 512] row
  per node) and AllGathers them in TWO chunks (node-range halves) so (a) the
  second half collective overlaps the first half's producing compute and
  (b) every gathered table has < 32768 rows -> int16 dma_gather indices.
- Edge phase per dst-block: ONE batched dma_gather per (table, block) pulls
  all K|V rows for the block's edges (994ns fixed + 0.34ns/row vs ~1.1us per
  128 rows with indirect_dma_start), one more for Q rows. Dot products /
  softmax / weighting run as block-batched bf16 DVE ops; segment-sum via a
  PE matmul per 128-edge tile with an is_equal selection matrix (bf16,
  fast-weight-load), accumulated in PSUM.
- Post phase fused into the same block loop: Wo/Wm/residual/LayerNorm and
  next layer's QKV run right after each block drains, keeping PE warm and
  h^T resident in SBUF. LayerNorm rsqrt = Exp(-0.5*Ln(var+eps)) so the ACT
  table set (natural_log_exp) never swaps.
- All matmuls bf16 (fp32 runs 2-pass LOW_HIGH at ~4x the cost), fp32 only
  for h state / PSUM accumulation / softmax sums.
"""
import sys

if "/opt/trn_rl_repo" not in sys.path:
    sys.path.insert(0, "/opt/trn_rl_repo")

import numpy as np
import ml_dtypes

from concourse import bass, mybir, bacc, tile
from concourse.masks import make_identity
from concourse.bass_utils import run_bass_kernel_spmd

FP = mybir.dt.float32
BF = mybir.dt.bfloat16
I16 = mybir.dt.int16
AF = mybir.ActivationFunctionType
OP = mybir.AluOpType

P = 128
NCORES = 8
H = 256
NH = 8
HD = H // NH
ED = 3
F_IN = 10
L = 3
EPS_LN = 1e-5
HC = H // P            # feature chunks (2)
BF_NP = ml_dtypes.bfloat16


# ---------------------------------------------------------------- host prep

def edge_schedule(edge_index, N):
    """Per-core edge schedule with uniform (cross-core identical) tiling.

    Edges are owned by the core holding their dst. Within each dst-block of
    128 nodes the edges are split into two segments by src table half
    (A: src local row < ROWS_A_LOC, B: rest), each segment padded to whole
    128-edge tiles, tile counts maxed across cores so the single SPMD
    program fits all cores.
    """
    nsh = N // NCORES
    blk = (nsh + P - 1) // P
    blkA = (blk + 1) // 2
    rowsA_loc = blkA * P              # local rows in table A
    rowsB_loc = blk * P - rowsA_loc   # local rows in table B
    assert rowsA_loc * NCORES < 32768 and rowsB_loc * NCORES < 32768

    src = edge_index[0].astype(np.int64)
    dst = edge_index[1].astype(np.int64)
    core_of = dst // nsh
    scr = src // nsh                  # owning core of src
    srl = src % nsh                   # local row of src
    inA = srl < rowsA_loc
    rowA = scr * rowsA_loc + srl                  # row in table A
    rowB = scr * rowsB_loc + (srl - rowsA_loc)    # row in table B

    per_core = []
    nA = np.zeros((NCORES, blk), np.int64)
    nB = np.zeros((NCORES, blk), np.int64)
    for c in range(NCORES):
        eids = np.where(core_of == c)[0]
        ld = dst[eids] - c * nsh
        b = ld // P
        # order: block asc, then segment (A first), stable
        order = np.lexsort((~inA[eids], b))
        eids = eids[order]
        b = b[order]
        per_core.append((eids, b))
        for blki in range(blk):
            be = eids[b == blki]
            a = inA[be].sum()
            nA[c, blki] = a
            nB[c, blki] = len(be) - a
    TA = np.maximum(0, -(-nA.max(axis=0) // P)).astype(np.int64)
    TB = np.maximum(0, -(-nB.max(axis=0) // P)).astype(np.int64)
    # ensure at least one tile per block so the scatter matmul always runs
    zero = (TA + TB) == 0
    TA[zero] = 1
    T = TA + TB
    T_tot = int(T.sum())
    toff = np.concatenate([[0], np.cumsum(T)])[:-1]

    cores = []
    for c in range(NCORES):
        eids, b = per_core[c]
        idxA = np.zeros((blk, max(1, int(TA.max())) * P), np.int16)
        idxB = np.zeros((blk, max(1, int(TB.max())) * P), np.int16)
        idxQ = np.zeros((P, T_tot), np.int16)
        dl = np.full((P, T_tot), -1.0, np.float32)
        esel = np.full(T_tot * P, -1, np.int64)
        for blki in range(blk):
            be = eids[b == blki]
            a = int(inA[be].sum())
            beA, beB = be[:a], be[a:]
            # segment A -> slots [0, TA*P), segment B -> [TA*P, T*P)
            for seg, base_slot, idxarr, rowarr in (
                (beA, 0, idxA, rowA),
                (beB, int(TA[blki]) * P, idxB, rowB),
            ):
                n = len(seg)
                if n:
                    idxarr[blki, :n] = rowarr[seg].astype(np.int16)
                    j = base_slot + np.arange(n)
                    pp, tt = j % P, j // P
                    ldl = dst[seg] - c * nsh - blki * P
                    idxQ[pp, toff[blki] + tt] = (blki * P + ldl).astype(np.int16)
                    dl[pp, toff[blki] + tt] = ldl.astype(np.float32)
                    esel[(toff[blki] + tt) * P + pp] = seg
        cores.append(dict(idxA=idxA, idxB=idxB, idxQ=idxQ, dl=dl, esel=esel))
    meta = dict(nsh=nsh, blk=blk, blkA=blkA, rowsA=rowsA_loc, rowsB=rowsB_loc,
                TA=TA, TB=TB, T=T, toff=toff, T_tot=T_tot)
    return meta, cores


def wrap16(vals_int16, ntiles):
    """[ntiles*128] slot-ordered indices -> dma_gather layout [128, ntiles*8]:
    wrapped into 16 partitions and replicated across the 8 Q7-core stripes."""
    if ntiles == 0:
        return np.zeros((P, 0), np.int16)
    out = np.zeros((16, ntiles * 8), np.int16)
    j = np.arange(ntiles * P)
    out[j % 16, j // 16] = vals_int16[: ntiles * P]
    return np.tile(out, (8, 1))


def build_blob(meta, core, ebias_l):
    """Per-layer per-block packed int16 blob:
    [idxA TA*8 | idxB TB*8 | idxQ T*8 | dl T (bf16) | ebias T*8 (bf16)].
    Returns (blob [128, W_tot], per-block column offsets/widths)."""
    blk = meta["blk"]
    TA, TB, T, toff = meta["TA"], meta["TB"], meta["T"], meta["toff"]
    cols = []
    offs = []
    o = 0
    for b in range(blk):
        ta, tb, t = int(TA[b]), int(TB[b]), int(T[b])
        iA = wrap16(core["idxA"][b], ta)
        iB = wrap16(core["idxB"][b], tb)
        # idxQ is stored slot-ordered [P, T]; rewrap to 16-partition layout
        qs = np.zeros(t * P, np.int16)
        j = np.arange(t * P)
        qs[j] = core["idxQ"][j % P, toff[b] + j // P]
        iQ = wrap16(qs, t)
        dlb = core["dl"][:, toff[b]:toff[b] + t].astype(BF_NP).view(np.int16)
        eb = ebias_l[:, toff[b] * 8:(toff[b] + t) * 8].view(np.int16)
        blob_b = np.concatenate([iA, iB, iQ, dlb, eb], axis=1)
        cols.append(blob_b)
        offs.append((o, ta, tb, t))
        o += blob_b.shape[1]
    return np.concatenate(cols, axis=1), offs


# ------------------------------------------------------------- device build

def build_program(N, meta, blob_w, blob_offs):
    nsh = meta["nsh"]
    blk = meta["blk"]
    blkA = meta["blkA"]
    rowsA, rowsB = meta["rowsA"], meta["rowsB"]
    nsh_pad = blk * P
    Tmax = int(meta["T"].max())
    rg = [list(range(NCORES))]

    nc = bacc.Bacc("TRN2", target_bir_lowering=False, debug=False,
                   num_devices=NCORES)

    # ---- I/O (weights host-converted to bf16 where used as matmul operands)
    xT = nc.dram_tensor("xT", [F_IN, nsh_pad], BF, kind="ExternalInput")
    blob = [nc.dram_tensor(f"blob{l}", [P, blob_w], I16, kind="ExternalInput")
            for l in range(L)]
    iota_in = nc.dram_tensor("iota_in", [P, Tmax * P], BF, kind="ExternalInput")
    w_in = nc.dram_tensor("w_in", [F_IN, H], BF, kind="ExternalInput")
    wq = nc.dram_tensor("wq", [L, H, H], BF, kind="ExternalInput")
    wk = nc.dram_tensor("wk", [L, H, H], BF, kind="ExternalInput")
    wv = nc.dram_tensor("wv", [L, H, H], BF, kind="ExternalInput")
    wo = nc.dram_tensor("wo", [L, H, H], BF, kind="ExternalInput")
    wm = nc.dram_tensor("wm", [L, 2 * H, H], BF, kind="ExternalInput")
    wh1 = nc.dram_tensor("wh1", [H, P], BF, kind="ExternalInput")
    wh2 = nc.dram_tensor("wh2", [P, 1], BF, kind="ExternalInput")
    # bias/scale rows, fp32 (replicated on-chip).
    # rows_in: 0:b_in 1:bo0 2:bm0 3:g0 4:bo1 5:bm1 6:g1 7:bo2 8:bm2 9:g2
    # rows2:   0:beta0 1:beta1 2:beta2 3:bh1(cols 0:P) 4:bh2(col 0)
    rows_in = nc.dram_tensor("rows_in", [1, 10 * H], FP, kind="ExternalInput")
    rows2 = nc.dram_tensor("rows2", [1, 5 * H], FP, kind="ExternalInput")
    y = nc.dram_tensor("y", [nsh_pad, 1], FP, kind="ExternalOutput")

    with tile.TileContext(nc) as tc:
        with (
            tc.tile_pool(name="sbw", bufs=1) as sbw,       # persistent
            tc.tile_pool(name="sbd", bufs=2) as sbd,       # dense working tiles
            tc.tile_pool(name="sbg", bufs=2) as sbg,       # per-block edge tiles
            tc.tile_pool(name="dram", bufs=1, space="DRAM") as dram,
            tc.tile_pool(name="p_acc", bufs=2, space="PSUM") as p_acc,
            tc.tile_pool(name="p_big", bufs=2, space="PSUM") as p_big,
            tc.tile_pool(name="p_tr", bufs=2, space="PSUM") as p_tr,
        ):
            # ---- persistent SBUF constants
            identb = sbw.tile([P, P], BF)
            make_identity(nc, identb[:])
            iota_sb = sbw.tile([P, Tmax * P], BF)
            nc.sync.dma_start(iota_sb[:], iota_in[:])
            ones1 = sbw.tile([1, P], BF)
            nc.vector.memset(ones1[:], 1.0)
            eps_col = sbw.tile([P, 1], FP)
            nc.vector.memset(eps_col[:], EPS_LN)

            w_in_sb = sbw.tile([F_IN, H], BF)
            nc.sync.dma_start(w_in_sb[:], w_in[:])

            def load_chunks(t, l, n_chunks, tag):
                out = []
                for kc in range(n_chunks):
                    s = sbw.tile([P, t.shape[-1]], BF, name=f"{tag}{l}_{kc}",
                                 tag=f"{tag}{l}_{kc}")
                    if l is None:
                        nc.sync.dma_start(s[:], t[kc * P:(kc + 1) * P, :])
                    else:
                        nc.sync.dma_start(s[:], t[l, kc * P:(kc + 1) * P, :])
                    out.append(s)
                return out

            wq_sb = [load_chunks(wq, l, HC, "wq") for l in range(L)]
            wk_sb = [load_chunks(wk, l, HC, "wk") for l in range(L)]
            wv_sb = [load_chunks(wv, l, HC, "wv") for l in range(L)]
            wo_sb = [load_chunks(wo, l, HC, "wo") for l in range(L)]
            wm_sb = [load_chunks(wm, l, 2 * HC, "wm") for l in range(L)]
            wh1_sb = load_chunks(wh1, None, HC, "wh1")
            wh2_sb = sbw.tile([P, 1], BF)
            nc.sync.dma_start(wh2_sb[:], wh2[:])

            ones_f = sbw.tile([1, P], FP)
            nc.vector.memset(ones_f[:], 1.0)
            rows_sb = sbw.tile([1, 10 * H], FP)
            nc.sync.dma_start(rows_sb[:], rows_in[:])
            rows2_sb = sbw.tile([1, 5 * H], FP)
            nc.sync.dma_start(rows2_sb[:], rows2[:])

            # replicate bias/scale rows to [P, H] bf16 tiles via PE
            def replicate_row(i, tag):
                ps = p_big.tile([P, H], FP, name="pq", tag="pq")
                nc.tensor.matmul(ps[:], lhsT=ones_f[:],
                                 rhs=rows_sb[:, i * H:(i + 1) * H],
                                 start=True, stop=True)
                t = sbw.tile([P, H], BF, name=tag, tag=tag)
                nc.scalar.copy(t[:], ps[:])
                return t

            b_in_rep = replicate_row(0, "b_in_rep")
            bo_rep = [replicate_row(1 + 3 * l, f"bo_rep{l}") for l in range(L)]
            bm_rep = [replicate_row(2 + 3 * l, f"bm_rep{l}") for l in range(L)]
            gb_rep = [replicate_row(3 + 3 * l, f"g_rep{l}") for l in range(L)]

            def replicate_row2(i, tag):
                ps = p_big.tile([P, H], FP, name="pq", tag="pq")
                nc.tensor.matmul(ps[:], lhsT=ones_f[:],
                                 rhs=rows2_sb[:, i * H:(i + 1) * H],
                                 start=True, stop=True)
                t = sbw.tile([P, H], BF, name=tag, tag=tag)
                nc.scalar.copy(t[:], ps[:])
                return t

            bet_rep = [replicate_row2(l, f"bet_rep{l}") for l in range(L)]
            bh_rep = replicate_row2(3, "bh_rep")      # [:, 0:P] = bh1 replicated
            bh2_rep = replicate_row2(4, "bh2_rep")    # [:, 0:1] = bh2

            # resident h^T (bf16) chunks: [128, nsh_pad] each
            hT = [sbw.tile([P, nsh_pad], BF, name=f"hT{kc}", tag=f"hT{kc}")
                  for kc in range(HC)]

            # ---- internal DRAM
            hdr = dram.tile([nsh_pad, H], FP)
            qtab = dram.tile([nsh_pad, H], BF)
            kvcb = dram.tile([nsh_pad, 2 * H], BF)
            kvfullA = [dram.tile([NCORES * rowsA, 2 * H], BF, addr_space="Shared",
                                 name=f"kvfa{l}", tag=f"kvfa{l}") for l in range(L)]
            kvfullB = [dram.tile([NCORES * rowsB, 2 * H], BF, addr_space="Shared",
                                 name=f"kvfb{l}", tag=f"kvfb{l}") for l in range(L)]

            # ---------------- helpers
            def transpose_cp(dst_bf_ap, src_bf_ap):
                pt = p_tr.tile([P, P], BF, name="ptr", tag="ptr")
                nc.tensor.transpose(out=pt[:], in_=src_bf_ap, identity=identb[:])
                nc.scalar.copy(dst_bf_ap, pt[:])

            def qkv_block(l, b, skip_kv=False):
                """Q/K/V for node block b of layer l from resident hT; write
                qtab / kvcb rows, return nothing."""
                q_ps = p_big.tile([P, H], FP, name="pq", tag="pq")
                kv_ps = p_big.tile([P, 2 * H], FP, name="pkv", tag="pkv")
                for kc in range(HC):
                    nc.tensor.matmul(q_ps[:], lhsT=hT[kc][:, b * P:(b + 1) * P],
                                     rhs=wq_sb[l][kc][:],
                                     start=(kc == 0), stop=(kc == HC - 1))
                # K then V as ONE accumulation group in one bank: start=True
                # only on the first matmul (bank-wide has_written clear), V
                # region elements are first-writes and overwrite cleanly.
                for kc in range(HC):
                    nc.tensor.matmul(kv_ps[:, 0:H],
                                     lhsT=hT[kc][:, b * P:(b + 1) * P],
                                     rhs=wk_sb[l][kc][:],
                                     start=(kc == 0), stop=False)
                for kc in range(HC):
                    nc.tensor.matmul(kv_ps[:, H:2 * H],
                                     lhsT=hT[kc][:, b * P:(b + 1) * P],
                                     rhs=wv_sb[l][kc][:],
                                     start=False, stop=(kc == HC - 1))
                qsb = sbd.tile([P, H], BF, name="qsb", tag="qsb")
                nc.scalar.copy(qsb[:], q_ps[:])
                nc.sync.dma_start(qtab[b * P:(b + 1) * P, :], qsb[:])
                kvsb = sbd.tile([P, 2 * H], BF, name="kvsb", tag="kvsb")
                nc.scalar.copy(kvsb[:], kv_ps[:])
                nc.sync.dma_start(kvcb[b * P:(b + 1) * P, :], kvsb[:])

            def allgather_chunks(l, b):
                if b == blkA - 1:
                    nc.gpsimd.collective_compute(
                        "AllGather", OP.bypass,
                        ins=[kvcb[0:rowsA, :].opt()],
                        outs=[kvfullA[l][:].opt()], replica_groups=rg)
                if b == blk - 1:
                    nc.gpsimd.collective_compute(
                        "AllGather", OP.bypass,
                        ins=[kvcb[rowsA:nsh_pad, :].opt()],
                        outs=[kvfullB[l][:].opt()], replica_groups=rg)

            def update_hT(l, b, h_bf_tile):
                for kc in range(HC):
                    transpose_cp(hT[kc][:, b * P:(b + 1) * P],
                                 h_bf_tile[:, kc * P:(kc + 1) * P])

            # ---------------- phase 0: input projection + QKV(0) + AG(0)
            for b in range(blk):
                xt = sbd.tile([F_IN, P], BF, name="xt", tag="xt")
                nc.sync.dma_start(xt[:], xT[:, b * P:(b + 1) * P])
                ps = p_big.tile([P, H], FP, name="pq", tag="pq")
                nc.tensor.matmul(ps[:], lhsT=xt[:], rhs=w_in_sb[:],
                                 start=True, stop=True)
                t0 = sbd.tile([P, H], FP, name="t0", tag="t0")
                nc.vector.tensor_tensor(t0[:], ps[:], b_in_rep[:], op=OP.add)
                h0 = sbd.tile([P, H], FP, name="h0", tag="h0")
                nc.scalar.activation(h0[:], t0[:], AF.Relu)
                nc.sync.dma_start(hdr[b * P:(b + 1) * P, :], h0[:])
                h0b = sbd.tile([P, H], BF, name="h0b", tag="h0b")
                nc.vector.tensor_copy(h0b[:], h0[:])
                update_hT(0, b, h0b)
                qkv_block(0, b)
                allgather_chunks(0, b)

            # ---------------- layers
            for l in range(L):
                last = (l == L - 1)
                for b in range(blk):
                    o, ta, tb, t = blob_offs[b]
                    w_b = ta * 8 + tb * 8 + t * 8 + t + t * 8
                    bl = sbg.tile([P, w_b], I16, name="bl", tag="bl")
                    nc.sync.dma_start(bl[:], blob[l][:, o:o + w_b])
                    oA, oB, oQ = 0, ta * 8, ta * 8 + tb * 8
                    oD, oE = oQ + t * 8, oQ + t * 8 + t

                    # dma_gather crashes above 1024 indices per call -> chunk
                    # at 8 tiles (the wrap16 idx layout is self-similar under
                    # 8-column slicing, so chunks just slice the idx region).
                    MAXT = 8

                    def gather(out3, tlo, nt, in_ap, icol, elem):
                        for c0 in range(0, nt, MAXT):
                            cn = min(MAXT, nt - c0)
                            nc.gpsimd.dma_gather(
                                out_ap=out3[:, tlo + c0:tlo + c0 + cn, :],
                                in_ap=in_ap,
                                idxs_ap=bl[:, icol + c0 * 8:icol + (c0 + cn) * 8],
                                num_idxs=cn * P, num_idxs_reg=cn * P,
                                elem_size=elem)

                    kvg = sbg.tile([P, t, 2 * H], BF, name="kvg", tag="kvg")
                    gather(kvg, 0, ta, kvfullA[l][:], oA, 2 * H)
                    gather(kvg, ta, tb, kvfullB[l][:], oB, 2 * H)
                    qg = sbg.tile([P, t, H], BF, name="qg", tag="qg")
                    gather(qg, 0, t, qtab[:], oQ, H)

                    # rhs buffer [P, t, 264] also serves as the qk scratch:
                    # qk product lands in [:, :, 0:H], is reduced to dots,
                    # then overwritten by V*aexp (Tile orders the WAR).
                    rhs = sbg.tile([P, t, H + NH], BF, name="rhs", tag="rhs")
                    nc.vector.tensor_tensor(
                        rhs[:, :, 0:H].rearrange("p t (h d) -> p t h d", d=HD),
                        qg[:, :, :].rearrange("p t (h d) -> p t h d", d=HD),
                        kvg[:, :, 0:H].rearrange("p t (h d) -> p t h d", d=HD),
                        op=OP.mult)
                    dots = sbg.tile([P, t * NH], FP, name="dots", tag="dots")
                    nc.vector.reduce_sum(
                        dots[:].rearrange("p (t h o) -> p t h o", h=NH, o=1),
                        rhs[:, :, 0:H].rearrange("p t (h d) -> p t h d", d=HD),
                        axis=mybir.AxisListType.X)
                    lg = sbg.tile([P, t * NH], FP, name="lg", tag="lg")
                    nc.vector.tensor_tensor(lg[:], dots[:],
                                            bl[:, oE:oE + t * 8].bitcast(BF),
                                            op=OP.add)
                    lg2 = sbg.tile([P, t * NH], FP, name="lg2", tag="lg2")
                    nc.vector.scalar_tensor_tensor(lg2[:], in0=lg[:], scalar=0.2,
                                                   in1=lg[:], op0=OP.mult,
                                                   op1=OP.max)
                    aexp = sbg.tile([P, t * NH], BF, name="aexp", tag="aexp")
                    nc.scalar.activation(aexp[:], lg2[:], AF.Exp)

                    # rhs = [V*aexp | aexp] interleaved [P, t, 264]
                    nc.vector.tensor_tensor(
                        rhs[:, :, 0:H].rearrange("p t (h d) -> p t h d", d=HD),
                        kvg[:, :, H:2 * H].rearrange("p t (h d) -> p t h d", d=HD),
                        aexp[:].rearrange("p (t h o) -> p t h o", h=NH, o=1)
                            .to_broadcast([P, t, NH, HD]),
                        op=OP.mult)
                    nc.vector.tensor_copy(
                        rhs[:, :, H:H + NH],
                        aexp[:].rearrange("p (t h) -> p t h", h=NH))

                    # m matrix [P, t*128] bf16
                    m = sbg.tile([P, t * P], BF, name="m", tag="m")
                    nc.vector.tensor_tensor(
                        m[:].rearrange("p (t d) -> p t d", d=P),
                        bl[:, oD:oD + t].bitcast(BF)
                            .rearrange("p (t o) -> p t o", o=1)
                            .to_broadcast([P, t, P]),
                        iota_sb[:, 0:t * P].rearrange("p (t d) -> p t d", d=P),
                        op=OP.is_equal)

                    acc = p_acc.tile([P, H + NH], FP, name="pacc", tag="pacc")
                    for ti in range(t):
                        nc.tensor.matmul(acc[:], lhsT=m[:, ti * P:(ti + 1) * P],
                                         rhs=rhs[:, ti, :],
                                         start=(ti == 0), stop=(ti == t - 1))

                    # drain + normalize
                    ssum = sbd.tile([P, NH], FP, name="ssum", tag="ssum")
                    nc.vector.tensor_scalar_max(ssum[:], acc[:, H:H + NH], 1e-12)
                    rs = sbd.tile([P, NH], FP, name="rs", tag="rs")
                    nc.vector.reciprocal(rs[:], ssum[:])
                    aggb = sbd.tile([P, H], BF, name="aggb", tag="aggb")
                    nc.vector.tensor_tensor(
                        aggb[:].rearrange("p (h d) -> p h d", d=HD),
                        acc[:, 0:H].rearrange("p (h d) -> p h d", d=HD),
                        rs[:].rearrange("p (h o) -> p h o", o=1)
                            .to_broadcast([P, NH, HD]),
                        op=OP.mult)

                    # post: Wo
                    aT = sbd.tile([P, H], BF, name="aT", tag="aT")
                    for kc in range(HC):
                        transpose_cp(aT[:, kc * P:(kc + 1) * P],
                                     aggb[:, kc * P:(kc + 1) * P])
                    wo_ps = p_big.tile([P, H], FP, name="pq", tag="pq")
                    for kc in range(HC):
                        nc.tensor.matmul(wo_ps[:], lhsT=aT[:, kc * P:(kc + 1) * P],
                                         rhs=wo_sb[l][kc][:],
                                         start=(kc == 0), stop=(kc == HC - 1))
                    awob = sbd.tile([P, H], BF, name="awob", tag="awob")
                    nc.vector.tensor_tensor(awob[:], wo_ps[:], bo_rep[l][:],
                                            op=OP.add)
                    awoT = sbd.tile([P, H], BF, name="awoT", tag="awoT")
                    for kc in range(HC):
                        transpose_cp(awoT[:, kc * P:(kc + 1) * P],
                                     awob[:, kc * P:(kc + 1) * P])

                    # Wm on [h | awo]
                    wm_ps = p_big.tile([P, H], FP, name="pq", tag="pq")
                    for kc in range(HC):
                        nc.tensor.matmul(wm_ps[:],
                                         lhsT=hT[kc][:, b * P:(b + 1) * P],
                                         rhs=wm_sb[l][kc][:],
                                         start=(kc == 0), stop=False)
                    for kc in range(HC):
                        nc.tensor.matmul(wm_ps[:],
                                         lhsT=awoT[:, kc * P:(kc + 1) * P],
                                         rhs=wm_sb[l][HC + kc][:],
                                         start=False, stop=(kc == HC - 1))
                    tm = sbd.tile([P, H], FP, name="tm", tag="tm")
                    nc.vector.tensor_tensor(tm[:], wm_ps[:], bm_rep[l][:],
                                            op=OP.add)
                    upd = sbd.tile([P, H], FP, name="upd", tag="upd")
                    nc.scalar.activation(upd[:], tm[:], AF.Relu)

                    # residual + LN
                    h_old = sbd.tile([P, H], FP, name="h_old", tag="h_old")
                    nc.sync.dma_start(h_old[:], hdr[b * P:(b + 1) * P, :])
                    resid = sbd.tile([P, H], FP, name="resid", tag="resid")
                    nc.vector.tensor_tensor(resid[:], h_old[:], upd[:], op=OP.add)
                    mu = sbd.tile([P, 1], FP, name="mu", tag="mu")
                    nc.vector.reduce_sum(mu[:], resid[:], axis=mybir.AxisListType.X)
                    mus = sbd.tile([P, 1], FP, name="mus", tag="mus")
                    nc.vector.tensor_scalar_mul(mus[:], mu[:], 1.0 / H)
                    cent = sbd.tile([P, H], FP, name="cent", tag="cent")
                    nc.vector.tensor_scalar_sub(cent[:], resid[:], mus[:])
                    sq = sbd.tile([P, H], FP, name="sq", tag="sq")
                    nc.vector.tensor_tensor(sq[:], cent[:], cent[:], op=OP.mult)
                    ssq = sbd.tile([P, 1], FP, name="ssq", tag="ssq")
                    nc.vector.reduce_sum(ssq[:], sq[:], axis=mybir.AxisListType.X)
                    lnv = sbd.tile([P, 1], FP, name="lnv", tag="lnv")
                    nc.scalar.activation(lnv[:], ssq[:], AF.Ln, bias=eps_col[:],
                                         scale=1.0 / H)
                    rstd = sbd.tile([P, 1], FP, name="rstd", tag="rstd")
                    nc.scalar.activation(rstd[:], lnv[:], AF.Exp, bias=0.0,
                                         scale=-0.5)
                    normed = sbd.tile([P, H], FP, name="normed", tag="normed")
                    nc.vector.tensor_scalar_mul(normed[:], cent[:], rstd[:])
                    hg = sbd.tile([P, H], FP, name="hg", tag="hg")
                    nc.vector.tensor_tensor(hg[:], normed[:], gb_rep[l][:],
                                            op=OP.mult)
                    h_new = sbd.tile([P, H], FP, name="h_new", tag="h_new")
                    nc.vector.tensor_tensor(h_new[:], hg[:], bet_rep[l][:],
                                            op=OP.add)

                    hb = sbd.tile([P, H], BF, name="hb", tag="hb")
                    nc.vector.tensor_copy(hb[:], h_new[:])
                    update_hT(l, b, hb)
                    if not last:
                        nc.sync.dma_start(hdr[b * P:(b + 1) * P, :], h_new[:])
                        qkv_block(l + 1, b)
                        allgather_chunks(l + 1, b)
                    else:
                        # output head
                        h1_ps = p_big.tile([P, P], FP, name="pq", tag="pq")
                        for kc in range(HC):
                            nc.tensor.matmul(h1_ps[:],
                                             lhsT=hT[kc][:, b * P:(b + 1) * P],
                                             rhs=wh1_sb[kc][:],
                                             start=(kc == 0), stop=(kc == HC - 1))
                        t1s = sbd.tile([P, P], FP, name="t1s", tag="t1s")
                        nc.vector.tensor_tensor(t1s[:], h1_ps[:],
                                                bh_rep[:, 0:P], op=OP.add)
                        t1 = sbd.tile([P, P], BF, name="t1", tag="t1")
                        nc.scalar.activation(t1[:], t1s[:], AF.Relu)
                        t1T = sbd.tile([P, P], BF, name="t1T", tag="t1T")
                        transpose_cp(t1T[:], t1[:])
                        y_ps = p_big.tile([P, 1], FP, name="pq", tag="pq")
                        nc.tensor.matmul(y_ps[:], lhsT=t1T[:], rhs=wh2_sb[:],
                                         start=True, stop=True)
                        yt = sbd.tile([P, 1], FP, name="yt", tag="yt")
                        nc.vector.tensor_tensor(yt[:], y_ps[:], bh2_rep[:, 0:1],
                                                op=OP.add)
                        nc.sync.dma_start(y[b * P:(b + 1) * P, :], yt[:])

    nc.compile()
    return nc


# ------------------------------------------------------------------ driver

def make_in_maps(inputs, meta, cores):
    N = inputs["x"].shape[0]
    nsh = meta["nsh"]
    blk = meta["blk"]
    nsh_pad = blk * P
    Tmax = int(meta["T"].max())
    T_tot = meta["T_tot"]
    x = np.asarray(inputs["x"], np.float32)
    edge_attr = np.asarray(inputs["edge_attr"], np.float32)
    We = np.asarray(inputs["We"], np.float32)
    scale = HD ** -0.5

    def bf(a):
        return np.ascontiguousarray(np.asarray(a, np.float32).astype(BF_NP))

    rows_in = np.zeros((10, H), np.float32)
    rows_in[0, :] = np.asarray(inputs["b_in"], np.float32)
    for l in range(L):
        rows_in[1 + 3 * l] = np.asarray(inputs["bo"], np.float32)[l]
        rows_in[2 + 3 * l] = np.asarray(inputs["bm"], np.float32)[l]
        rows_in[3 + 3 * l] = np.asarray(inputs["gamma"], np.float32)[l]
    rows_in = rows_in.reshape(1, 10 * H)
    rows2 = np.zeros((5, H), np.float32)
    for l in range(L):
        rows2[l] = np.asarray(inputs["beta"], np.float32)[l]
    rows2[3, 0:P] = np.asarray(inputs["b_h1"], np.float32)
    rows2[4, 0] = float(np.asarray(inputs["b_h2"], np.float32).reshape(-1)[0])
    rows2 = rows2.reshape(1, 5 * H)

    iota = np.tile(np.arange(P, dtype=np.float32)[None, :], (P, Tmax))

    common = {
        "iota_in": bf(iota),
        "w_in": bf(inputs["W_in"]),
        "wq": bf(np.asarray(inputs["Wq"], np.float32) * scale),
        "wk": bf(inputs["Wk"]),
        "wv": bf(inputs["Wv"]),
        "wo": bf(inputs["Wo"]),
        "wm": bf(inputs["Wm"]),
        "wh1": bf(inputs["W_h1"]),
        "wh2": bf(np.asarray(inputs["W_h2"], np.float32).reshape(P, 1)),
        "rows_in": rows_in,
        "rows2": rows2,
    }

    in_maps = []
    blob_offs = None
    for c in range(NCORES):
        core = cores[c]
        xT = np.zeros((F_IN, nsh_pad), np.float32)
        xT[:, :nsh] = x[c * nsh:(c + 1) * nsh].T
        esel = core["esel"]
        valid = esel >= 0
        m = dict(common)
        m["xT"] = bf(xT)
        for l in range(L):
            eb = np.zeros((T_tot * P, NH), np.float32)
            eb[valid] = edge_attr[esel[valid]] @ We[l]
            # slot j=(tt*P+pp) -> ebias[pp, tt*8+h]
            ebias = np.ascontiguousarray(
                eb.reshape(T_tot, P, NH).transpose(1, 0, 2).reshape(P, T_tot * NH)
            ).astype(BF_NP)
            blob_arr, offs = build_blob(meta, core, ebias)
            m[f"blob{l}"] = blob_arr
            blob_offs = offs
        in_maps.append(m)
    return in_maps, blob_offs


_BUILD_CACHE = {}
LAST_EXEC_NS = None


def kernel(**inputs) -> np.ndarray:
    global LAST_EXEC_NS
    import os
    edge_index = np.asarray(inputs["edge_index"])
    N = inputs["x"].shape[0]
    nsh = N // NCORES
    meta, cores = edge_schedule(edge_index, N)
    in_maps, blob_offs = make_in_maps(inputs, meta, cores)
    blob_w = in_maps[0]["blob0"].shape[1]
    key = (N, blob_w, tuple(meta["T"].tolist()), tuple(meta["TA"].tolist()))
    if key not in _BUILD_CACHE:
        _BUILD_CACHE[key] = build_program(N, meta, blob_w, blob_offs)
    nc = _BUILD_CACHE[key]
    trace = os.environ.get("KERNEL_TRACE", "0") == "1"
    res = run_bass_kernel_spmd(nc, in_maps, core_ids=list(range(NCORES)),
                               trace=trace)
    if res.exec_time_ns is not None:
        LAST_EXEC_NS = res.exec_time_ns
        tp = res.instructions_and_trace[1] if res.instructions_and_trace else None
        print(f"[kernel] exec_time_ns={res.exec_time_ns} trace={tp}")
    out = np.concatenate([np.asarray(res.results[c]["y"])[:nsh]
                          for c in range(NCORES)], 0)
    return out.astype(np.float32)


# ---------------------------------------------------------------- reference

def np_forward(inp):
    """Numpy port of the jax reference (for --sim self-check)."""
    N = inp["x"].shape[0]
    src = inp["edge_index"][0].astype(np.int64)
    dst = inp["edge_index"][1].astype(np.int64)
    scale = HD ** -0.5
    h = np.maximum(inp["x"] @ inp["W_in"] + inp["b_in"], 0.0)
    for l in range(L):
        Q = (h @ inp["Wq"][l]).reshape(N, NH, HD)
        K = (h @ inp["Wk"][l]).reshape(N, NH, HD)
        V = (h @ inp["Wv"][l]).reshape(N, NH, HD)
        eb = inp["edge_attr"] @ inp["We"][l]
        attn = (Q[dst] * K[src]).sum(-1) * scale + eb
        attn = np.where(attn > 0, attn, 0.2 * attn)
        aexp = np.exp(attn - attn.max())
        asum = np.zeros((N, NH))
        np.add.at(asum, dst, aexp)
        anorm = aexp / np.clip(asum[dst], 1e-12, None)
        wV = V[src] * anorm[..., None]
        agg = np.zeros((N, NH, HD))
        np.add.at(agg, dst, wV)
        agg = agg.reshape(N, H) @ inp["Wo"][l] + inp["bo"][l]
        upd = np.maximum(
            np.concatenate([h, agg], 1) @ inp["Wm"][l] + inp["bm"][l], 0.0)
        hh = h + upd
        mu = hh.mean(-1, keepdims=True)
        var = hh.var(-1, keepdims=True)
        h = (hh - mu) / np.sqrt(var + EPS_LN) * inp["gamma"][l] + inp["beta"][l]
    return np.maximum(h @ inp["W_h1"] + inp["b_h1"], 0.0) @ inp["W_h2"] + inp["b_h2"]


if __name__ == "__main__":
    import argparse
    parser = argparse.ArgumentParser()
    parser.add_argument("--sim", action="store_true")
    args = parser.parse_args()
    if args.sim:
        from concourse.bass_interp import MultiCoreSim
        rng = np.random.default_rng(0)
        Nl, El = 2048, 16384
        g = lambda *s: (rng.standard_normal(s) * 0.05).astype(np.float32)
        inp = {
            "x": rng.standard_normal((Nl, F_IN)).astype(np.float32),
            "edge_attr": rng.standard_normal((El, ED)).astype(np.float32),
            "W_in": g(F_IN, H), "b_in": (rng.standard_normal(H) * 0.01).astype(np.float32),
            "Wq": g(L, H, H), "Wk": g(L, H, H), "Wv": g(L, H, H),
            "We": g(L, ED, NH), "Wo": g(L, H, H),
            "bo": (rng.standard_normal((L, H)) * 0.01).astype(np.float32),
            "Wm": g(L, 2 * H, H),
            "bm": (rng.standard_normal((L, H)) * 0.01).astype(np.float32),
            "gamma": (1 + 0.1 * rng.standard_normal((L, H))).astype(np.float32),
            "beta": (0.1 * rng.standard_normal((L, H))).astype(np.float32),
            "W_h1": g(H, H // 2), "b_h1": (rng.standard_normal(H // 2) * 0.01).astype(np.float32),
            "W_h2": g(H // 2, 1), "b_h2": np.zeros(1, np.float32),
            "edge_index": rng.integers(0, Nl, size=(2, El)).astype(np.int64),
        }
        meta, cores = edge_schedule(inp["edge_index"], Nl)
        in_maps, blob_offs = make_in_maps(inp, meta, cores)
        blob_w = in_maps[0]["blob0"].shape[1]
        print(f"sim build: T={meta['T'].tolist()} blob_w={blob_w}")
        nc = build_program(Nl, meta, blob_w, blob_offs)
        sim = MultiCoreSim(nc, num_cores=NCORES, num_workers=0)
        for c in range(NCORES):
            for k, v in in_maps[c].items():
                sim.cores[c].tensor(k)[:] = v
        sim.simulate(check_with_hw=False)
        nshl = Nl // NCORES
        got = np.concatenate([np.asarray(sim.cores[c].tensor("y"))[:nshl]
                              for c in range(NCORES)], 0)
        want = np_forward(inp)
        err = np.abs(got - want)
        print(f"sim maxabs={err.max():.3e} rel={err.max()/np.abs(want).max():.3e}")


# revision 22
# speedup vs baseline: 1.4551x; 1.0209x over previous
"""Trainium2 Bass kernel for AttentionProlongationGNN (optimized).

Contract: kernel(**inputs) takes FULL unsharded numpy inputs (keys as in
setup_inputs) and returns the FULL (N, 1) float32 output.

Strategy (8 NeuronCores, SPMD single program):
- Nodes sharded 6250/core (padded to 6272 = 49*128 rows).  Edges sharded by
  dst core, grouped into 49 dst-blocks of 128 nodes.
- Per layer each core computes K|V for its shard (bf16, one [nsh, 512] row
  per node) and AllGathers them in TWO chunks (node-range halves) so (a) the
  second half collective overlaps the first half's producing compute and
  (b) every gathered table has < 32768 rows -> int16 dma_gather indices.
- Edge phase per dst-block: ONE batched dma_gather per (table, block) pulls
  all K|V rows for the block's edges (994ns fixed + 0.34ns/row vs ~1.1us per
  128 rows with indirect_dma_start), one more for Q rows. Dot products /
  softmax / weighting run as block-batched bf16 DVE ops; segment-sum via a
  PE matmul per 128-edge tile with an is_equal selection matrix (bf16,
  fast-weight-load), accumulated in PSUM.
- Post phase fused into the same block loop: Wo/Wm/residual/LayerNorm and
  next layer's QKV run right after each block drains, keeping PE warm and
  h^T resident in SBUF. LayerNorm rsqrt = Exp(-0.5*Ln(var+eps)) so the ACT
  table set (natural_log_exp) never swaps.
- All matmuls bf16 (fp32 runs 2-pass LOW_HIGH at ~4x the cost), fp32 only
  for h state / PSUM accumulation / softmax sums.
"""
import sys

if "/opt/trn_rl_repo" not in sys.path:
    sys.path.insert(0, "/opt/trn_rl_repo")

import numpy as np
import ml_dtypes

from concourse import bass, mybir, bacc, tile
from concourse.masks import make_identity
from concourse.bass_utils import run_bass_kernel_spmd

FP = mybir.dt.float32
BF = mybir.dt.bfloat16
I16 = mybir.dt.int16
AF = mybir.ActivationFunctionType
OP = mybir.AluOpType

P = 128
NCORES = 8
H = 256
NH = 8
HD = H // NH
ED = 3
F_IN = 10
L = 3
EPS_LN = 1e-5
HC = H // P            # feature chunks (2)
BF_NP = ml_dtypes.bfloat16


# ---------------------------------------------------------------- host prep

def edge_schedule(edge_index, N):
    """Per-core edge schedule with uniform (cross-core identical) tiling.

    Edges are owned by the core holding their dst. Within each dst-block of
    128 nodes the edges are split into two segments by src table half
    (A: src local row < ROWS_A_LOC, B: rest), each segment padded to whole
    128-edge tiles, tile counts maxed across cores so the single SPMD
    program fits all cores.
    """
    nsh = N // NCORES
    blk = (nsh + P - 1) // P
    blkA = (blk + 1) // 2
    rowsA_loc = blkA * P              # local rows in table A
    rowsB_loc = blk * P - rowsA_loc   # local rows in table B
    assert rowsA_loc * NCORES < 32768 and rowsB_loc * NCORES < 32768

    src = edge_index[0].astype(np.int64)
    dst = edge_index[1].astype(np.int64)
    core_of = dst // nsh
    scr = src // nsh                  # owning core of src
    srl = src % nsh                   # local row of src
    inA = srl < rowsA_loc
    rowA = scr * rowsA_loc + srl                  # row in table A
    rowB = scr * rowsB_loc + (srl - rowsA_loc)    # row in table B

    per_core = []
    nA = np.zeros((NCORES, blk), np.int64)
    nB = np.zeros((NCORES, blk), np.int64)
    for c in range(NCORES):
        eids = np.where(core_of == c)[0]
        ld = dst[eids] - c * nsh
        b = ld // P
        # order: block asc, then segment (A first), stable
        order = np.lexsort((~inA[eids], b))
        eids = eids[order]
        b = b[order]
        per_core.append((eids, b))
        for blki in range(blk):
            be = eids[b == blki]
            a = inA[be].sum()
            nA[c, blki] = a
            nB[c, blki] = len(be) - a
    TA = np.maximum(0, -(-nA.max(axis=0) // P)).astype(np.int64)
    TB = np.maximum(0, -(-nB.max(axis=0) // P)).astype(np.int64)
    # ensure at least one tile per block so the scatter matmul always runs
    zero = (TA + TB) == 0
    TA[zero] = 1
    T = TA + TB
    T_tot = int(T.sum())
    toff = np.concatenate([[0], np.cumsum(T)])[:-1]

    cores = []
    for c in range(NCORES):
        eids, b = per_core[c]
        idxA = np.zeros((blk, max(1, int(TA.max())) * P), np.int16)
        idxB = np.zeros((blk, max(1, int(TB.max())) * P), np.int16)
        idxQ = np.zeros((P, T_tot), np.int16)
        dl = np.full((P, T_tot), -1.0, np.float32)
        esel = np.full(T_tot * P, -1, np.int64)
        for blki in range(blk):
            be = eids[b == blki]
            a = int(inA[be].sum())
            beA, beB = be[:a], be[a:]
            # segment A -> slots [0, TA*P), segment B -> [TA*P, T*P)
            for seg, base_slot, idxarr, rowarr in (
                (beA, 0, idxA, rowA),
                (beB, int(TA[blki]) * P, idxB, rowB),
            ):
                n = len(seg)
                if n:
                    idxarr[blki, :n] = rowarr[seg].astype(np.int16)
                    j = base_slot + np.arange(n)
                    pp, tt = j % P, j // P
                    ldl = dst[seg] - c * nsh - blki * P
                    idxQ[pp, toff[blki] + tt] = (blki * P + ldl).astype(np.int16)
                    dl[pp, toff[blki] + tt] = ldl.astype(np.float32)
                    esel[(toff[blki] + tt) * P + pp] = seg
        cores.append(dict(idxA=idxA, idxB=idxB, idxQ=idxQ, dl=dl, esel=esel))
    meta = dict(nsh=nsh, blk=blk, blkA=blkA, rowsA=rowsA_loc, rowsB=rowsB_loc,
                TA=TA, TB=TB, T=T, toff=toff, T_tot=T_tot)
    return meta, cores


def wrap16(vals_int16, ntiles):
    """[ntiles*128] slot-ordered indices -> dma_gather layout [128, ntiles*8]:
    wrapped into 16 partitions and replicated across the 8 Q7-core stripes."""
    if ntiles == 0:
        return np.zeros((P, 0), np.int16)
    out = np.zeros((16, ntiles * 8), np.int16)
    j = np.arange(ntiles * P)
    out[j % 16, j // 16] = vals_int16[: ntiles * P]
    return np.tile(out, (8, 1))


def build_blob(meta, core, ebias_l):
    """Per-layer per-block packed int16 blob:
    [idxA TA*8 | idxB TB*8 | idxQ T*8 | dl T (bf16) | ebias T*8 (bf16)].
    Returns (blob [128, W_tot], per-block column offsets/widths)."""
    blk = meta["blk"]
    TA, TB, T, toff = meta["TA"], meta["TB"], meta["T"], meta["toff"]
    cols = []
    offs = []
    o = 0
    for b in range(blk):
        ta, tb, t = int(TA[b]), int(TB[b]), int(T[b])
        iA = wrap16(core["idxA"][b], ta)
        iB = wrap16(core["idxB"][b], tb)
        # idxQ is stored slot-ordered [P, T]; rewrap to 16-partition layout
        qs = np.zeros(t * P, np.int16)
        j = np.arange(t * P)
        qs[j] = core["idxQ"][j % P, toff[b] + j // P]
        iQ = wrap16(qs, t)
        dlb = core["dl"][:, toff[b]:toff[b] + t].astype(BF_NP).view(np.int16)
        eb = ebias_l[:, toff[b] * 8:(toff[b] + t) * 8].view(np.int16)
        blob_b = np.concatenate([iA, iB, iQ, dlb, eb], axis=1)
        cols.append(blob_b)
        offs.append((o, ta, tb, t))
        o += blob_b.shape[1]
    return np.concatenate(cols, axis=1), offs


# ------------------------------------------------------------- device build

def build_program(N, meta, blob_w, blob_offs):
    nsh = meta["nsh"]
    blk = meta["blk"]
    blkA = meta["blkA"]
    rowsA, rowsB = meta["rowsA"], meta["rowsB"]
    nsh_pad = blk * P
    Tmax = int(meta["T"].max())
    rg = [list(range(NCORES))]

    nc = bacc.Bacc("TRN2", target_bir_lowering=False, debug=False,
                   num_devices=NCORES)

    # ---- I/O (weights host-converted to bf16 where used as matmul operands)
    xT = nc.dram_tensor("xT", [F_IN, nsh_pad], BF, kind="ExternalInput")
    blob = [nc.dram_tensor(f"blob{l}", [P, blob_w], I16, kind="ExternalInput")
            for l in range(L)]
    iota_in = nc.dram_tensor("iota_in", [P, Tmax * P], BF, kind="ExternalInput")
    w_in = nc.dram_tensor("w_in", [F_IN, H], BF, kind="ExternalInput")
    wq = nc.dram_tensor("wq", [L, H, H], BF, kind="ExternalInput")
    wk = nc.dram_tensor("wk", [L, H, H], BF, kind="ExternalInput")
    wv = nc.dram_tensor("wv", [L, H, H], BF, kind="ExternalInput")
    wo = nc.dram_tensor("wo", [L, H, H], BF, kind="ExternalInput")
    wm = nc.dram_tensor("wm", [L, 2 * H, H], BF, kind="ExternalInput")
    wh1 = nc.dram_tensor("wh1", [H, P], BF, kind="ExternalInput")
    wh2 = nc.dram_tensor("wh2", [P, 1], BF, kind="ExternalInput")
    # bias/scale rows, fp32 (replicated on-chip).
    # rows_in: 0:b_in 1:bo0 2:bm0 3:g0 4:bo1 5:bm1 6:g1 7:bo2 8:bm2 9:g2
    # rows2:   0:beta0 1:beta1 2:beta2 3:bh1(cols 0:P) 4:bh2(col 0)
    rows_in = nc.dram_tensor("rows_in", [1, 10 * H], FP, kind="ExternalInput")
    rows2 = nc.dram_tensor("rows2", [1, 5 * H], FP, kind="ExternalInput")
    y = nc.dram_tensor("y", [nsh_pad, 1], FP, kind="ExternalOutput")

    with tile.TileContext(nc) as tc:
        with (
            tc.tile_pool(name="sbw", bufs=1) as sbw,       # persistent
            tc.tile_pool(name="sbd", bufs=2) as sbd,       # dense working tiles
            tc.tile_pool(name="sbg", bufs=2) as sbg,       # per-block edge tiles
            tc.tile_pool(name="dram", bufs=1, space="DRAM") as dram,
            tc.tile_pool(name="p_acc", bufs=2, space="PSUM") as p_acc,
            tc.tile_pool(name="p_big", bufs=2, space="PSUM") as p_big,
            tc.tile_pool(name="p_tr", bufs=2, space="PSUM") as p_tr,
        ):
            # ---- persistent SBUF constants
            identb = sbw.tile([P, P], BF)
            make_identity(nc, identb[:])
            iota_sb = sbw.tile([P, Tmax * P], BF)
            nc.sync.dma_start(iota_sb[:], iota_in[:])
            ones1 = sbw.tile([1, P], BF)
            nc.vector.memset(ones1[:], 1.0)
            eps_col = sbw.tile([P, 1], FP)
            nc.vector.memset(eps_col[:], EPS_LN)

            w_in_sb = sbw.tile([F_IN, H], BF)
            nc.sync.dma_start(w_in_sb[:], w_in[:])

            def load_chunks(t, l, n_chunks, tag):
                out = []
                for kc in range(n_chunks):
                    s = sbw.tile([P, t.shape[-1]], BF, name=f"{tag}{l}_{kc}",
                                 tag=f"{tag}{l}_{kc}")
                    if l is None:
                        nc.sync.dma_start(s[:], t[kc * P:(kc + 1) * P, :])
                    else:
                        nc.sync.dma_start(s[:], t[l, kc * P:(kc + 1) * P, :])
                    out.append(s)
                return out

            wq_sb = [load_chunks(wq, l, HC, "wq") for l in range(L)]
            wk_sb = [load_chunks(wk, l, HC, "wk") for l in range(L)]
            wv_sb = [load_chunks(wv, l, HC, "wv") for l in range(L)]
            wo_sb = [load_chunks(wo, l, HC, "wo") for l in range(L)]
            wm_sb = [load_chunks(wm, l, 2 * HC, "wm") for l in range(L)]
            wh1_sb = load_chunks(wh1, None, HC, "wh1")
            wh2_sb = sbw.tile([P, 1], BF)
            nc.sync.dma_start(wh2_sb[:], wh2[:])

            ones_f = sbw.tile([1, P], FP)
            nc.vector.memset(ones_f[:], 1.0)
            rows_sb = sbw.tile([1, 10 * H], FP)
            nc.sync.dma_start(rows_sb[:], rows_in[:])
            rows2_sb = sbw.tile([1, 5 * H], FP)
            nc.sync.dma_start(rows2_sb[:], rows2[:])

            # replicate bias/scale rows to [P, H] bf16 tiles via PE
            def replicate_row(i, tag):
                ps = p_big.tile([P, H], FP, name="pq", tag="pq")
                nc.tensor.matmul(ps[:], lhsT=ones_f[:],
                                 rhs=rows_sb[:, i * H:(i + 1) * H],
                                 start=True, stop=True)
                t = sbw.tile([P, H], BF, name=tag, tag=tag)
                nc.scalar.copy(t[:], ps[:])
                return t

            b_in_rep = replicate_row(0, "b_in_rep")
            bo_rep = [replicate_row(1 + 3 * l, f"bo_rep{l}") for l in range(L)]
            bm_rep = [replicate_row(2 + 3 * l, f"bm_rep{l}") for l in range(L)]
            gb_rep = [replicate_row(3 + 3 * l, f"g_rep{l}") for l in range(L)]

            def replicate_row2(i, tag):
                ps = p_big.tile([P, H], FP, name="pq", tag="pq")
                nc.tensor.matmul(ps[:], lhsT=ones_f[:],
                                 rhs=rows2_sb[:, i * H:(i + 1) * H],
                                 start=True, stop=True)
                t = sbw.tile([P, H], BF, name=tag, tag=tag)
                nc.scalar.copy(t[:], ps[:])
                return t

            bet_rep = [replicate_row2(l, f"bet_rep{l}") for l in range(L)]
            bh_rep = replicate_row2(3, "bh_rep")      # [:, 0:P] = bh1 replicated
            bh2_rep = replicate_row2(4, "bh2_rep")    # [:, 0:1] = bh2

            # resident h^T (bf16) chunks: [128, nsh_pad] each
            hT = [sbw.tile([P, nsh_pad], BF, name=f"hT{kc}", tag=f"hT{kc}")
                  for kc in range(HC)]

            # ---- internal DRAM
            hdr = dram.tile([nsh_pad, H], FP)
            qtab = dram.tile([nsh_pad, H], BF)
            kvcb = dram.tile([nsh_pad, 2 * H], BF)
            kvfullA = [dram.tile([NCORES * rowsA, 2 * H], BF, addr_space="Shared",
                                 name=f"kvfa{l}", tag=f"kvfa{l}") for l in range(L)]
            kvfullB = [dram.tile([NCORES * rowsB, 2 * H], BF, addr_space="Shared",
                                 name=f"kvfb{l}", tag=f"kvfb{l}") for l in range(L)]

            # ---------------- helpers
            def transpose_cp(dst_bf_ap, src_bf_ap):
                pt = p_tr.tile([P, P], BF, name="ptr", tag="ptr")
                nc.tensor.transpose(out=pt[:], in_=src_bf_ap, identity=identb[:])
                nc.scalar.copy(dst_bf_ap, pt[:])

            def qkv_block(l, b, skip_kv=False):
                """Q/K/V for node block b of layer l from resident hT; write
                qtab / kvcb rows, return nothing."""
                q_ps = p_big.tile([P, H], FP, name="pq", tag="pq")
                kv_ps = p_big.tile([P, 2 * H], FP, name="pkv", tag="pkv")
                for kc in range(HC):
                    nc.tensor.matmul(q_ps[:], lhsT=hT[kc][:, b * P:(b + 1) * P],
                                     rhs=wq_sb[l][kc][:],
                                     start=(kc == 0), stop=(kc == HC - 1))
                # K then V as ONE accumulation group in one bank: start=True
                # only on the first matmul (bank-wide has_written clear), V
                # region elements are first-writes and overwrite cleanly.
                for kc in range(HC):
                    nc.tensor.matmul(kv_ps[:, 0:H],
                                     lhsT=hT[kc][:, b * P:(b + 1) * P],
                                     rhs=wk_sb[l][kc][:],
                                     start=(kc == 0), stop=False)
                for kc in range(HC):
                    nc.tensor.matmul(kv_ps[:, H:2 * H],
                                     lhsT=hT[kc][:, b * P:(b + 1) * P],
                                     rhs=wv_sb[l][kc][:],
                                     start=False, stop=(kc == HC - 1))
                qsb = sbd.tile([P, H], BF, name="qsb", tag="qsb")
                nc.scalar.copy(qsb[:], q_ps[:])
                nc.sync.dma_start(qtab[b * P:(b + 1) * P, :], qsb[:])
                kvsb = sbd.tile([P, 2 * H], BF, name="kvsb", tag="kvsb")
                nc.scalar.copy(kvsb[:], kv_ps[:])
                nc.sync.dma_start(kvcb[b * P:(b + 1) * P, :], kvsb[:])

            def allgather_chunks(l, b):
                if b == blkA - 1:
                    nc.gpsimd.collective_compute(
                        "AllGather", OP.bypass,
                        ins=[kvcb[0:rowsA, :].opt()],
                        outs=[kvfullA[l][:].opt()], replica_groups=rg)
                if b == blk - 1:
                    nc.gpsimd.collective_compute(
                        "AllGather", OP.bypass,
                        ins=[kvcb[rowsA:nsh_pad, :].opt()],
                        outs=[kvfullB[l][:].opt()], replica_groups=rg)

            def update_hT(l, b, h_bf_tile):
                for kc in range(HC):
                    transpose_cp(hT[kc][:, b * P:(b + 1) * P],
                                 h_bf_tile[:, kc * P:(kc + 1) * P])

            # ---------------- phase 0: input projection + QKV(0) + AG(0)
            for b in range(blk):
                xt = sbd.tile([F_IN, P], BF, name="xt", tag="xt")
                nc.sync.dma_start(xt[:], xT[:, b * P:(b + 1) * P])
                ps = p_big.tile([P, H], FP, name="pq", tag="pq")
                nc.tensor.matmul(ps[:], lhsT=xt[:], rhs=w_in_sb[:],
                                 start=True, stop=True)
                t0 = sbd.tile([P, H], FP, name="t0", tag="t0")
                nc.vector.tensor_tensor(t0[:], ps[:], b_in_rep[:], op=OP.add)
                h0 = sbd.tile([P, H], FP, name="h0", tag="h0")
                nc.scalar.activation(h0[:], t0[:], AF.Relu)
                nc.sync.dma_start(hdr[b * P:(b + 1) * P, :], h0[:])
                h0b = sbd.tile([P, H], BF, name="h0b", tag="h0b")
                nc.vector.tensor_copy(h0b[:], h0[:])
                update_hT(0, b, h0b)
                qkv_block(0, b)
                allgather_chunks(0, b)

            # ---------------- layers
            for l in range(L):
                last = (l == L - 1)
                for b in range(blk):
                    o, ta, tb, t = blob_offs[b]
                    w_b = ta * 8 + tb * 8 + t * 8 + t + t * 8
                    bl = sbg.tile([P, w_b], I16, name="bl", tag="bl")
                    nc.sync.dma_start(bl[:], blob[l][:, o:o + w_b])
                    oA, oB, oQ = 0, ta * 8, ta * 8 + tb * 8
                    oD, oE = oQ + t * 8, oQ + t * 8 + t

                    # dma_gather crashes above 1024 indices per call -> chunk
                    # into near-equal pieces of <= 8 tiles (the wrap16 idx
                    # layout is self-similar under 8-column slicing).
                    def gather(out3, tlo, nt, in_ap, icol, elem):
                        nch = -(-nt // 8)
                        done = 0
                        for c in range(nch):
                            cn = (nt - done + nch - c - 1) // (nch - c)
                            c0 = done
                            done += cn
                            nc.gpsimd.dma_gather(
                                out_ap=out3[:, tlo + c0:tlo + c0 + cn, :],
                                in_ap=in_ap,
                                idxs_ap=bl[:, icol + c0 * 8:icol + (c0 + cn) * 8],
                                num_idxs=cn * P, num_idxs_reg=cn * P,
                                elem_size=elem)

                    kvg = sbg.tile([P, t, 2 * H], BF, name="kvg", tag="kvg")
                    gather(kvg, 0, ta, kvfullA[l][:], oA, 2 * H)
                    gather(kvg, ta, tb, kvfullB[l][:], oB, 2 * H)
                    qg = sbg.tile([P, t, H], BF, name="qg", tag="qg")
                    gather(qg, 0, t, qtab[:], oQ, H)

                    # wvq [P, t*256] is qk scratch first, then V*aexp.
                    # (Stride-0 broadcasts drop DVE to 1 elem/cycle; plain
                    # strided APs keep the 2x bf16 mode, so everything here
                    # avoids broadcast operands on DVE.)
                    wvq = sbg.tile([P, t * H], BF, name="wvq", tag="wvq")
                    nc.vector.tensor_tensor(
                        wvq[:], qg[:, :, :], kvg[:, :, 0:H], op=OP.mult)
                    dots = sbg.tile([P, t * NH], BF, name="dots", tag="dots")
                    with nc.allow_low_precision("attn logits fit bf16"):
                        nc.vector.reduce_sum(
                            dots[:].rearrange("p (g o) -> p g o", o=1),
                            wvq[:].rearrange("p (g d) -> p g d", d=HD),
                            axis=mybir.AxisListType.X)
                    lg = sbg.tile([P, t * NH], FP, name="lg", tag="lg")
                    nc.vector.tensor_tensor(lg[:], dots[:],
                                            bl[:, oE:oE + t * 8].bitcast(BF),
                                            op=OP.add)
                    lg2 = sbg.tile([P, t * NH], FP, name="lg2", tag="lg2")
                    nc.vector.scalar_tensor_tensor(lg2[:], in0=lg[:], scalar=0.2,
                                                   in1=lg[:], op0=OP.mult,
                                                   op1=OP.max)
                    # exp expanded across head dim on ACT (stride-0 read is
                    # cheap there), into the qg buffer (qk already consumed it)
                    wvexp = qg[:].rearrange("p t (h d) -> p (t h) d", d=HD)
                    nc.scalar.activation(
                        wvexp, lg2[:].rearrange("p (g o) -> p g o", o=1)
                        .to_broadcast([P, t * NH, HD]), AF.Exp)
                    aexp8 = sbg.tile([P, t * NH], BF, name="aexp8", tag="aexp8")
                    nc.scalar.activation(aexp8[:], lg2[:], AF.Exp)

                    nc.vector.tensor_tensor(
                        wvq[:], kvg[:, :, H:2 * H], wvexp, op=OP.mult)

                    # m matrix [P, t*128] bf16
                    m = sbg.tile([P, t * P], BF, name="m", tag="m")
                    nc.vector.tensor_tensor(
                        m[:].rearrange("p (t d) -> p t d", d=P),
                        bl[:, oD:oD + t].bitcast(BF)
                            .rearrange("p (t o) -> p t o", o=1)
                            .to_broadcast([P, t, P]),
                        iota_sb[:, 0:t * P].rearrange("p (t d) -> p t d", d=P),
                        op=OP.is_equal)

                    acc = p_acc.tile([P, H + NH], FP, name="pacc", tag="pacc")
                    for ti in range(t):
                        nc.tensor.matmul(acc[:, 0:H],
                                         lhsT=m[:, ti * P:(ti + 1) * P],
                                         rhs=wvq[:, ti * H:(ti + 1) * H],
                                         start=(ti == 0), stop=False)
                        nc.tensor.matmul(acc[:, H:H + NH],
                                         lhsT=m[:, ti * P:(ti + 1) * P],
                                         rhs=aexp8[:, ti * NH:(ti + 1) * NH],
                                         start=False, stop=(ti == t - 1))

                    # drain + normalize
                    ssum = sbd.tile([P, NH], FP, name="ssum", tag="ssum")
                    nc.vector.tensor_scalar_max(ssum[:], acc[:, H:H + NH], 1e-12)
                    rs = sbd.tile([P, NH], FP, name="rs", tag="rs")
                    nc.vector.reciprocal(rs[:], ssum[:])
                    aggb = sbd.tile([P, H], BF, name="aggb", tag="aggb")
                    nc.vector.tensor_tensor(
                        aggb[:].rearrange("p (h d) -> p h d", d=HD),
                        acc[:, 0:H].rearrange("p (h d) -> p h d", d=HD),
                        rs[:].rearrange("p (h o) -> p h o", o=1)
                            .to_broadcast([P, NH, HD]),
                        op=OP.mult)

                    # post: Wo
                    aT = sbd.tile([P, H], BF, name="aT", tag="aT")
                    for kc in range(HC):
                        transpose_cp(aT[:, kc * P:(kc + 1) * P],
                                     aggb[:, kc * P:(kc + 1) * P])
                    wo_ps = p_big.tile([P, H], FP, name="pq", tag="pq")
                    for kc in range(HC):
                        nc.tensor.matmul(wo_ps[:], lhsT=aT[:, kc * P:(kc + 1) * P],
                                         rhs=wo_sb[l][kc][:],
                                         start=(kc == 0), stop=(kc == HC - 1))
                    awob = sbd.tile([P, H], BF, name="awob", tag="awob")
                    nc.vector.tensor_tensor(awob[:], wo_ps[:], bo_rep[l][:],
                                            op=OP.add)
                    awoT = sbd.tile([P, H], BF, name="awoT", tag="awoT")
                    for kc in range(HC):
                        transpose_cp(awoT[:, kc * P:(kc + 1) * P],
                                     awob[:, kc * P:(kc + 1) * P])

                    # Wm on [h | awo]
                    wm_ps = p_big.tile([P, H], FP, name="pq", tag="pq")
                    for kc in range(HC):
                        nc.tensor.matmul(wm_ps[:],
                                         lhsT=hT[kc][:, b * P:(b + 1) * P],
                                         rhs=wm_sb[l][kc][:],
                                         start=(kc == 0), stop=False)
                    for kc in range(HC):
                        nc.tensor.matmul(wm_ps[:],
                                         lhsT=awoT[:, kc * P:(kc + 1) * P],
                                         rhs=wm_sb[l][HC + kc][:],
                                         start=False, stop=(kc == HC - 1))
                    tm = sbd.tile([P, H], FP, name="tm", tag="tm")
                    nc.vector.tensor_tensor(tm[:], wm_ps[:], bm_rep[l][:],
                                            op=OP.add)
                    upd = sbd.tile([P, H], FP, name="upd", tag="upd")
                    nc.scalar.activation(upd[:], tm[:], AF.Relu)

                    # residual + LN
                    h_old = sbd.tile([P, H], FP, name="h_old", tag="h_old")
                    nc.sync.dma_start(h_old[:], hdr[b * P:(b + 1) * P, :])
                    resid = sbd.tile([P, H], FP, name="resid", tag="resid")
                    nc.vector.tensor_tensor(resid[:], h_old[:], upd[:], op=OP.add)
                    mu = sbd.tile([P, 1], FP, name="mu", tag="mu")
                    nc.vector.reduce_sum(mu[:], resid[:], axis=mybir.AxisListType.X)
                    mus = sbd.tile([P, 1], FP, name="mus", tag="mus")
                    nc.vector.tensor_scalar_mul(mus[:], mu[:], 1.0 / H)
                    # var = E[x^2] - mu^2 (no centered tensor materialised)
                    r2 = sbd.tile([P, H], FP, name="r2", tag="r2")
                    nc.vector.tensor_tensor(r2[:], resid[:], resid[:], op=OP.mult)
                    ssq = sbd.tile([P, 1], FP, name="ssq", tag="ssq")
                    nc.vector.reduce_sum(ssq[:], r2[:], axis=mybir.AxisListType.X)
                    mu2 = sbd.tile([P, 1], FP, name="mu2", tag="mu2")
                    nc.vector.tensor_tensor(mu2[:], mus[:], mus[:], op=OP.mult)
                    vpe = sbd.tile([P, 1], FP, name="vpe", tag="vpe")
                    nc.vector.scalar_tensor_tensor(vpe[:], in0=ssq[:],
                                                   scalar=1.0 / H, in1=mu2[:],
                                                   op0=OP.mult, op1=OP.subtract)
                    lnv = sbd.tile([P, 1], FP, name="lnv", tag="lnv")
                    nc.scalar.activation(lnv[:], vpe[:], AF.Ln, bias=eps_col[:])
                    rstd = sbd.tile([P, 1], FP, name="rstd", tag="rstd")
                    nc.scalar.activation(rstd[:], lnv[:], AF.Exp, bias=0.0,
                                         scale=-0.5)
                    nmr = sbd.tile([P, 1], FP, name="nmr", tag="nmr")
                    nc.vector.scalar_tensor_tensor(nmr[:], in0=mus[:],
                                                   scalar=-1.0, in1=rstd[:],
                                                   op0=OP.mult, op1=OP.mult)
                    # normed = (resid - mu) * rstd fused on ACT
                    normed = sbd.tile([P, H], FP, name="normed", tag="normed")
                    nc.scalar.activation(normed[:], resid[:], AF.Identity,
                                         bias=nmr[:], scale=rstd[:])
                    hg = sbd.tile([P, H], FP, name="hg", tag="hg")
                    nc.vector.tensor_tensor(hg[:], normed[:], gb_rep[l][:],
                                            op=OP.mult)
                    h_new = sbd.tile([P, H], FP, name="h_new", tag="h_new")
                    nc.vector.tensor_tensor(h_new[:], hg[:], bet_rep[l][:],
                                            op=OP.add)

                    hb = sbd.tile([P, H], BF, name="hb", tag="hb")
                    nc.vector.tensor_copy(hb[:], h_new[:])
                    update_hT(l, b, hb)
                    if not last:
                        nc.sync.dma_start(hdr[b * P:(b + 1) * P, :], h_new[:])
                        qkv_block(l + 1, b)
                        allgather_chunks(l + 1, b)
                    else:
                        # output head
                        h1_ps = p_big.tile([P, P], FP, name="pq", tag="pq")
                        for kc in range(HC):
                            nc.tensor.matmul(h1_ps[:],
                                             lhsT=hT[kc][:, b * P:(b + 1) * P],
                                             rhs=wh1_sb[kc][:],
                                             start=(kc == 0), stop=(kc == HC - 1))
                        t1s = sbd.tile([P, P], FP, name="t1s", tag="t1s")
                        nc.vector.tensor_tensor(t1s[:], h1_ps[:],
                                                bh_rep[:, 0:P], op=OP.add)
                        t1 = sbd.tile([P, P], BF, name="t1", tag="t1")
                        nc.scalar.activation(t1[:], t1s[:], AF.Relu)
                        t1T = sbd.tile([P, P], BF, name="t1T", tag="t1T")
                        transpose_cp(t1T[:], t1[:])
                        y_ps = p_big.tile([P, 1], FP, name="pq", tag="pq")
                        nc.tensor.matmul(y_ps[:], lhsT=t1T[:], rhs=wh2_sb[:],
                                         start=True, stop=True)
                        yt = sbd.tile([P, 1], FP, name="yt", tag="yt")
                        nc.vector.tensor_tensor(yt[:], y_ps[:], bh2_rep[:, 0:1],
                                                op=OP.add)
                        nc.sync.dma_start(y[b * P:(b + 1) * P, :], yt[:])

    nc.compile()
    return nc


# ------------------------------------------------------------------ driver

def make_in_maps(inputs, meta, cores):
    N = inputs["x"].shape[0]
    nsh = meta["nsh"]
    blk = meta["blk"]
    nsh_pad = blk * P
    Tmax = int(meta["T"].max())
    T_tot = meta["T_tot"]
    x = np.asarray(inputs["x"], np.float32)
    edge_attr = np.asarray(inputs["edge_attr"], np.float32)
    We = np.asarray(inputs["We"], np.float32)
    scale = HD ** -0.5

    def bf(a):
        return np.ascontiguousarray(np.asarray(a, np.float32).astype(BF_NP))

    rows_in = np.zeros((10, H), np.float32)
    rows_in[0, :] = np.asarray(inputs["b_in"], np.float32)
    for l in range(L):
        rows_in[1 + 3 * l] = np.asarray(inputs["bo"], np.float32)[l]
        rows_in[2 + 3 * l] = np.asarray(inputs["bm"], np.float32)[l]
        rows_in[3 + 3 * l] = np.asarray(inputs["gamma"], np.float32)[l]
    rows_in = rows_in.reshape(1, 10 * H)
    rows2 = np.zeros((5, H), np.float32)
    for l in range(L):
        rows2[l] = np.asarray(inputs["beta"], np.float32)[l]
    rows2[3, 0:P] = np.asarray(inputs["b_h1"], np.float32)
    rows2[4, 0] = float(np.asarray(inputs["b_h2"], np.float32).reshape(-1)[0])
    rows2 = rows2.reshape(1, 5 * H)

    iota = np.tile(np.arange(P, dtype=np.float32)[None, :], (P, Tmax))

    common = {
        "iota_in": bf(iota),
        "w_in": bf(inputs["W_in"]),
        "wq": bf(np.asarray(inputs["Wq"], np.float32) * scale),
        "wk": bf(inputs["Wk"]),
        "wv": bf(inputs["Wv"]),
        "wo": bf(inputs["Wo"]),
        "wm": bf(inputs["Wm"]),
        "wh1": bf(inputs["W_h1"]),
        "wh2": bf(np.asarray(inputs["W_h2"], np.float32).reshape(P, 1)),
        "rows_in": rows_in,
        "rows2": rows2,
    }

    in_maps = []
    blob_offs = None
    for c in range(NCORES):
        core = cores[c]
        xT = np.zeros((F_IN, nsh_pad), np.float32)
        xT[:, :nsh] = x[c * nsh:(c + 1) * nsh].T
        esel = core["esel"]
        valid = esel >= 0
        m = dict(common)
        m["xT"] = bf(xT)
        for l in range(L):
            eb = np.zeros((T_tot * P, NH), np.float32)
            eb[valid] = edge_attr[esel[valid]] @ We[l]
            # slot j=(tt*P+pp) -> ebias[pp, tt*8+h]
            ebias = np.ascontiguousarray(
                eb.reshape(T_tot, P, NH).transpose(1, 0, 2).reshape(P, T_tot * NH)
            ).astype(BF_NP)
            blob_arr, offs = build_blob(meta, core, ebias)
            m[f"blob{l}"] = blob_arr
            blob_offs = offs
        in_maps.append(m)
    return in_maps, blob_offs


_BUILD_CACHE = {}
LAST_EXEC_NS = None


def kernel(**inputs) -> np.ndarray:
    global LAST_EXEC_NS
    import os
    edge_index = np.asarray(inputs["edge_index"])
    N = inputs["x"].shape[0]
    nsh = N // NCORES
    meta, cores = edge_schedule(edge_index, N)
    in_maps, blob_offs = make_in_maps(inputs, meta, cores)
    blob_w = in_maps[0]["blob0"].shape[1]
    key = (N, blob_w, tuple(meta["T"].tolist()), tuple(meta["TA"].tolist()))
    if key not in _BUILD_CACHE:
        _BUILD_CACHE[key] = build_program(N, meta, blob_w, blob_offs)
    nc = _BUILD_CACHE[key]
    trace = os.environ.get("KERNEL_TRACE", "0") == "1"
    res = run_bass_kernel_spmd(nc, in_maps, core_ids=list(range(NCORES)),
                               trace=trace)
    if res.exec_time_ns is not None:
        LAST_EXEC_NS = res.exec_time_ns
        tp = res.instructions_and_trace[1] if res.instructions_and_trace else None
        print(f"[kernel] exec_time_ns={res.exec_time_ns} trace={tp}")
    out = np.concatenate([np.asarray(res.results[c]["y"])[:nsh]
                          for c in range(NCORES)], 0)
    return out.astype(np.float32)


# ---------------------------------------------------------------- reference

def np_forward(inp):
    """Numpy port of the jax reference (for --sim self-check)."""
    N = inp["x"].shape[0]
    src = inp["edge_index"][0].astype(np.int64)
    dst = inp["edge_index"][1].astype(np.int64)
    scale = HD ** -0.5
    h = np.maximum(inp["x"] @ inp["W_in"] + inp["b_in"], 0.0)
    for l in range(L):
        Q = (h @ inp["Wq"][l]).reshape(N, NH, HD)
        K = (h @ inp["Wk"][l]).reshape(N, NH, HD)
        V = (h @ inp["Wv"][l]).reshape(N, NH, HD)
        eb = inp["edge_attr"] @ inp["We"][l]
        attn = (Q[dst] * K[src]).sum(-1) * scale + eb
        attn = np.where(attn > 0, attn, 0.2 * attn)
        aexp = np.exp(attn - attn.max())
        asum = np.zeros((N, NH))
        np.add.at(asum, dst, aexp)
        anorm = aexp / np.clip(asum[dst], 1e-12, None)
        wV = V[src] * anorm[..., None]
        agg = np.zeros((N, NH, HD))
        np.add.at(agg, dst, wV)
        agg = agg.reshape(N, H) @ inp["Wo"][l] + inp["bo"][l]
        upd = np.maximum(
            np.concatenate([h, agg], 1) @ inp["Wm"][l] + inp["bm"][l], 0.0)
        hh = h + upd
        mu = hh.mean(-1, keepdims=True)
        var = hh.var(-1, keepdims=True)
        h = (hh - mu) / np.sqrt(var + EPS_LN) * inp["gamma"][l] + inp["beta"][l]
    return np.maximum(h @ inp["W_h1"] + inp["b_h1"], 0.0) @ inp["W_h2"] + inp["b_h2"]


if __name__ == "__main__":
    import argparse
    parser = argparse.ArgumentParser()
    parser.add_argument("--sim", action="store_true")
    args = parser.parse_args()
    if args.sim:
        from concourse.bass_interp import MultiCoreSim
        rng = np.random.default_rng(0)
        Nl, El = 2048, 16384
        g = lambda *s: (rng.standard_normal(s) * 0.05).astype(np.float32)
        inp = {
            "x": rng.standard_normal((Nl, F_IN)).astype(np.float32),
            "edge_attr": rng.standard_normal((El, ED)).astype(np.float32),
            "W_in": g(F_IN, H), "b_in": (rng.standard_normal(H) * 0.01).astype(np.float32),
            "Wq": g(L, H, H), "Wk": g(L, H, H), "Wv": g(L, H, H),
            "We": g(L, ED, NH), "Wo": g(L, H, H),
            "bo": (rng.standard_normal((L, H)) * 0.01).astype(np.float32),
            "Wm": g(L, 2 * H, H),
            "bm": (rng.standard_normal((L, H)) * 0.01).astype(np.float32),
            "gamma": (1 + 0.1 * rng.standard_normal((L, H))).astype(np.float32),
            "beta": (0.1 * rng.standard_normal((L, H))).astype(np.float32),
            "W_h1": g(H, H // 2), "b_h1": (rng.standard_normal(H // 2) * 0.01).astype(np.float32),
            "W_h2": g(H // 2, 1), "b_h2": np.zeros(1, np.float32),
            "edge_index": rng.integers(0, Nl, size=(2, El)).astype(np.int64),
        }
        meta, cores = edge_schedule(inp["edge_index"], Nl)
        in_maps, blob_offs = make_in_maps(inp, meta, cores)
        blob_w = in_maps[0]["blob0"].shape[1]
        print(f"sim build: T={meta['T'].tolist()} blob_w={blob_w}")
        nc = build_program(Nl, meta, blob_w, blob_offs)
        sim = MultiCoreSim(nc, num_cores=NCORES, num_workers=0)
        for c in range(NCORES):
            for k, v in in_maps[c].items():
                sim.cores[c].tensor(k)[:] = v
        sim.simulate(check_with_hw=False)
        nshl = Nl // NCORES
        got = np.concatenate([np.asarray(sim.cores[c].tensor("y"))[:nshl]
                              for c in range(NCORES)], 0)
        want = np_forward(inp)
        err = np.abs(got - want)
        print(f"sim maxabs={err.max():.3e} rel={err.max()/np.abs(want).max():.3e}")
